# revision 1
# baseline (speedup 1.0000x reference)
"""Chamfer distance (L1) Trainium2 Bass kernel.

Problem: xyz1 (4, 8192, 3) fp32, xyz2 (4, 8192, 3) fp32 ->
scalar = mean_b[ mean_n min_m ||x1-x2|| + mean_m min_n ||x1-x2|| ].

Strategy:
 - 8 cores: core c handles batch b=c//2, N-half h=c%2 -> a (4096 x 8192)
   distance block per core.
 - d2[n,m] = ||x1n||^2 + ||x2m||^2 - 2 x1n.x2m is computed as ONE matmul with
   an augmented contraction dim: K=33 rows of 3-level split-precision bf16
   (x = hi+mid+lo, all 9 cross products + 3-way split norms), giving ~fp32
   accuracy at bf16 PE speed (1 cyc/row vs 4 for fp32). Rows are ordered so
   PSUM partial sums stay small (cancellation early).
 - sqrt is monotone: min(sqrt(max(d2,0))) = sqrt(max(min(d2),0)), so sqrt and
   means happen on host over only 12K values per core.
 - ScalarE (ACT) drains each PSUM chunk to SBUF as fp16 scaled by 2^14 (free
   scale on the activation path; scaling keeps tiny d2 out of fp16
   subnormals, and overflow->inf is harmless under min).
 - VectorE does both min directions as fp16 tensor_tensor(min) folds in 2x
   mode: row-direction (over m) into rowacc + small reduce per n-tile;
   col-direction (over n) into a [128, 8192] accumulator, finished with PE
   transposes + free-axis reduces.
"""

import sys

sys.path.insert(0, "/opt/trn_rl_repo")

import numpy as np
import ml_dtypes

import concourse.bass as bass
import concourse.bacc as bacc
import concourse.mybir as mybir
import concourse.tile as tile
from concourse.bass_utils import run_bass_kernel_spmd

BF16 = mybir.dt.bfloat16
FP16 = mybir.dt.float16
FP32 = mybir.dt.float32
NP_BF16 = ml_dtypes.bfloat16

B, N, M = 4, 8192, 8192
N_CORES = 8
NC_N = N // 2  # 4096 rows per core
K_AUG = 33
D2_SCALE = 512.0  # 2^9: keeps d2*scale in fp16 normal range (max ~100*512 < 65504)

N_TILES = NC_N // 128  # 32
CHUNK = 2048  # psum chunk free size (4 matmuls of 512)
M_CHUNKS = M // CHUNK  # 4


def build_program():
    nc = bacc.Bacc()

    lhs_d = nc.dram_tensor("lhs", [K_AUG, NC_N], BF16, kind="ExternalInput").ap()
    rhs_d = nc.dram_tensor("rhs", [K_AUG, M], BF16, kind="ExternalInput").ap()
    ident_d = nc.dram_tensor("ident", [128, 128], FP16, kind="ExternalInput").ap()
    rowmin_d = nc.dram_tensor(
        "rowmin", [128, N_TILES], FP32, kind="ExternalOutput"
    ).ap()
    colmin_d = nc.dram_tensor(
        "colmin", [128, M // 128], FP32, kind="ExternalOutput"
    ).ap()

    amin = mybir.AluOpType.min
    ax_x = mybir.AxisListType.X

    with tile.TileContext(nc) as tc:
        with (
            tc.tile_pool(name="const", bufs=1) as const_pool,
            tc.tile_pool(name="acc", bufs=1) as acc_pool,
            tc.tile_pool(name="row", bufs=3) as row_pool,
            tc.tile_pool(name="drain", bufs=4) as drain_pool,
            tc.tile_pool(name="out", bufs=1) as out_pool,
            tc.tile_pool(name="mm", bufs=2, space="PSUM") as mm_pool,
        ):
            lhs_sb = const_pool.tile([K_AUG, NC_N], BF16)
            rhs_sb = const_pool.tile([K_AUG, M], BF16)
            ident_sb = const_pool.tile([128, 128], FP16)
            nc.sync.dma_start(out=lhs_sb, in_=lhs_d)
            nc.sync.dma_start(out=rhs_sb, in_=rhs_d)
            nc.sync.dma_start(out=ident_sb, in_=ident_d)

            colacc = acc_pool.tile([128, M], FP16)  # fold over n-tiles
            rowmin_sb = out_pool.tile([128, N_TILES], FP32)
            colmin_sb = out_pool.tile([128, M // 128], FP32)

            for i in range(N_TILES):
                lhs_i = lhs_sb[:, i * 128 : (i + 1) * 128]
                rowacc = row_pool.tile([128, CHUNK], FP16)
                for jp in range(M_CHUNKS // 2):
                    # drain a PAIR of psum chunks into one [128,4096] tile so
                    # the col-direction fold runs as one wide 4096 op
                    pair = drain_pool.tile([128, 2 * CHUNK], FP16)
                    for half in range(2):
                        jg = jp * 2 + half
                        psum_t = mm_pool.tile([128, CHUNK], FP32, tag="mm")
                        for q in range(CHUNK // 512):
                            j = jg * (CHUNK // 512) + q
                            nc.tensor.matmul(
                                psum_t[:, q * 512 : (q + 1) * 512],
                                lhs_i,
                                rhs_sb[:, j * 512 : (j + 1) * 512],
                            )
                        # ACT drains PSUM -> SBUF fp16 with free *D2_SCALE
                        nc.scalar.mul(
                            pair[:, half * CHUNK : (half + 1) * CHUNK],
                            psum_t,
                            D2_SCALE,
                        )
                    # row-direction fold (over m), fp16 2x mode
                    if jp == 0:
                        nc.vector.tensor_tensor(
                            rowacc, pair[:, :CHUNK], pair[:, CHUNK:], amin
                        )
                    else:
                        nc.vector.tensor_tensor(rowacc, rowacc, pair[:, :CHUNK], amin)
                        nc.vector.tensor_tensor(rowacc, rowacc, pair[:, CHUNK:], amin)
                    # col-direction fold (over n), one wide fp16 2x op
                    cslice = colacc[:, jp * 2 * CHUNK : (jp + 1) * 2 * CHUNK]
                    if i == 0:
                        nc.vector.tensor_copy(cslice, pair)
                    else:
                        nc.vector.tensor_tensor(cslice, cslice, pair, amin)
                # finish row-direction for this n-tile: halve 3x, then reduce
                nc.vector.tensor_tensor(
                    rowacc[:, : CHUNK // 2],
                    rowacc[:, : CHUNK // 2],
                    rowacc[:, CHUNK // 2 :],
                    amin,
                )
                nc.vector.tensor_tensor(
                    rowacc[:, : CHUNK // 4],
                    rowacc[:, : CHUNK // 4],
                    rowacc[:, CHUNK // 4 : CHUNK // 2],
                    amin,
                )
                nc.vector.tensor_tensor(
                    rowacc[:, : CHUNK // 8],
                    rowacc[:, : CHUNK // 8],
                    rowacc[:, CHUNK // 8 : CHUNK // 4],
                    amin,
                )
                nc.vector.tensor_reduce(
                    rowmin_sb[:, i : i + 1],
                    rowacc[:, : CHUNK // 8],
                    axis=ax_x,
                    op=amin,
                )

            # clamp so a stray inf can't become NaN via the transpose matmul
            nc.vector.tensor_scalar_min(colacc, colacc, 60000.0)
            # finish col-direction: transpose 128-wide chunks (4 per PSUM tile),
            # then one fused free-axis min per group of 4
            for g in range(M // 512):
                tr_t = mm_pool.tile([128, 512], FP16, tag="mm")
                for c4 in range(4):
                    cc = g * 4 + c4
                    nc.tensor.transpose(
                        tr_t[:, c4 * 128 : (c4 + 1) * 128],
                        colacc[:, cc * 128 : (cc + 1) * 128],
                        ident_sb,
                    )
                nc.vector.tensor_reduce(
                    colmin_sb[:, g * 4 : (g + 1) * 4],
                    tr_t.rearrange("p (a b) -> p a b", b=128),
                    axis=ax_x,
                    op=amin,
                )

            nc.sync.dma_start(out=rowmin_d, in_=rowmin_sb)
            nc.sync.dma_start(out=colmin_d, in_=colmin_sb)

    nc.compile()
    return nc


def _split3(v):
    """v (f64 array) -> (hi, mid, lo) bf16 with hi+mid+lo ~= v (~26-bit)."""
    v = v.astype(np.float64)
    hi = v.astype(NP_BF16)
    r1 = v - hi.astype(np.float64)
    mid = r1.astype(NP_BF16)
    lo = (r1 - mid.astype(np.float64)).astype(NP_BF16)
    return hi, mid, lo


def _make_core_inputs(x1h, x2):
    """x1h (4096,3), x2 (8192,3) fp32 -> lhs [33,4096], rhs [33,8192] bf16.

    Row pairing (lhs_k paired with rhs_k), ordered so PE partial sums cancel
    early: d2 = sq1 + sq2 - 2*x1.x2 with 3-level splits.
    """
    x1h = x1h.astype(np.float64)
    x2 = x2.astype(np.float64)
    a1 = _split3(x1h)  # (hi, mid, lo), each (4096, 3)
    a2 = _split3(x2)
    n2 = [(-2.0 * p.astype(np.float64)).astype(NP_BF16) for p in a2]  # exact *-2
    sq1 = (x1h * x1h).sum(-1)
    sq2 = (x2 * x2).sum(-1)
    s1 = _split3(sq1)
    s2 = _split3(sq2)

    ones_n = np.ones(NC_N, NP_BF16)
    ones_m = np.ones(M, NP_BF16)

    lhs_rows = []
    rhs_rows = []

    def add(l, r):
        lhs_rows.append(l)
        rhs_rows.append(r)

    # big terms first, interleaved for cancellation
    add(s1[0], ones_m)
    for d in range(3):
        add(a1[0][:, d], n2[0][:, d])  # hi*hi
    add(ones_n, s2[0])
    # mid-level terms
    add(s1[1], ones_m)
    add(ones_n, s2[1])
    for d in range(3):
        add(a1[0][:, d], n2[1][:, d])  # hi*mid
    for d in range(3):
        add(a1[1][:, d], n2[0][:, d])  # mid*hi
    for d in range(3):
        add(a1[1][:, d], n2[1][:, d])  # mid*mid
    # low-level terms
    add(s1[2], ones_m)
    add(ones_n, s2[2])
    for d in range(3):
        add(a1[0][:, d], n2[2][:, d])  # hi*lo
    for d in range(3):
        add(a1[2][:, d], n2[0][:, d])  # lo*hi
    for d in range(3):
        add(a1[1][:, d], n2[2][:, d])  # mid*lo
    for d in range(3):
        add(a1[2][:, d], n2[1][:, d])  # lo*mid
    for d in range(3):
        add(a1[2][:, d], n2[2][:, d])  # lo*lo

    lhs = np.ascontiguousarray(np.stack(lhs_rows))
    rhs = np.ascontiguousarray(np.stack(rhs_rows))
    assert lhs.shape == (K_AUG, NC_N) and rhs.shape == (K_AUG, M)
    return lhs, rhs


_CACHED_NC = None


def _get_nc():
    global _CACHED_NC
    if _CACHED_NC is None:
        _CACHED_NC = build_program()
    return _CACHED_NC


def kernel(xyz1, xyz2, _return_timing=False, _trace=False):
    xyz1 = np.asarray(xyz1, dtype=np.float32)
    xyz2 = np.asarray(xyz2, dtype=np.float32)
    assert xyz1.shape == (B, N, 3) and xyz2.shape == (B, M, 3)

    ident = np.eye(128, dtype=np.float16)
    in_maps = []
    for c in range(N_CORES):
        b, h = divmod(c, 2)
        lhs, rhs = _make_core_inputs(xyz1[b, h * NC_N : (h + 1) * NC_N], xyz2[b])
        in_maps.append({"lhs": lhs, "rhs": rhs, "ident": ident})

    nc = _get_nc()
    res = run_bass_kernel_spmd(
        nc, in_maps, core_ids=list(range(N_CORES)), trace=_trace
    )

    total = 0.0
    for b in range(B):
        row_parts = []
        col_parts = []
        for h in range(2):
            r = res.results[2 * b + h]
            row_parts.append(
                np.asarray(r["rowmin"]).astype(np.float64).T.reshape(-1)
            )  # (4096,)
            col_parts.append(
                np.asarray(r["colmin"]).astype(np.float64).T.reshape(-1)
            )  # (8192,)
        min1_d2 = np.concatenate(row_parts) / D2_SCALE  # (8192,)
        min2_d2 = np.minimum(col_parts[0], col_parts[1]) / D2_SCALE  # (8192,)
        min1 = np.sqrt(np.maximum(min1_d2, 0.0))
        min2 = np.sqrt(np.maximum(min2_d2, 0.0))
        total += min1.mean() + min2.mean()
    out = np.asarray(total / B, dtype=np.float32)
    if _return_timing:
        return out, res
    return out



# revision 6
# speedup vs baseline: 5.6334x; 5.6334x over previous
"""Chamfer distance (L1) Trainium2 Bass kernel — sorted-window version.

Problem: xyz1 (4, 8192, 3) fp32, xyz2 (4, 8192, 3) fp32 ->
scalar = mean_b[ mean_n min_m ||x1-x2|| + mean_m min_n ||x1-x2|| ].

Strategy:
 - 8 cores: core c handles batch b=c//2, N-half h=c%2 (4096 rows).
 - Host sorts both clouds by x. A 128-row tile of sorted rows only needs
   distances against a W-column rank-window around its aligned position:
   the device computes a banded (windowed) distance matrix instead of the
   full 4096x8192 block — W/8192 of the dense work.
 - Rigorous exactness: a windowed row/col min is provably exact whenever
   it is <= the x-gap to the nearest excluded column/row (|dx| lower-bounds
   the distance). The host flags the few points (~0.5%) violating this and
   recomputes them exactly in numpy. Result: exact up to fp16 rounding.
 - d2[n,m] = ||x1n||^2 + ||x2m||^2 - 2 x1n.x2m as ONE matmul with K=33 rows
   of 3-level split-precision bf16 (~fp32 accuracy at bf16 PE speed).
 - All minimums run as MAX over NEGATED distances (ACT drains PSUM with a
   free *-1): max-folds on DVE for both directions, and the partition
   direction (col mins) finishes on GpSimd's partition_all_reduce(max) —
   no PE transposes needed.
 - Per 2-tile pair: PE matmuls -> PSUM [128,2W]; ACT drains -d2 to fp16;
   DVE: one [128,2,W/2] max-halving into a wide rowhalf buffer (log-folded
   in deferred groups) + one strided max-fold into colacc per tile.
 - sqrt + means + flag-fix on host over ~12K values per core.
"""

import sys

sys.path.insert(0, "/opt/trn_rl_repo")

import numpy as np
import ml_dtypes

import concourse.bass as bass
import concourse.bass_isa as bass_isa
import concourse.bacc as bacc
import concourse.mybir as mybir
import concourse.tile as tile
from concourse.bass_utils import run_bass_kernel_spmd

BF16 = mybir.dt.bfloat16
FP16 = mybir.dt.float16
FP32 = mybir.dt.float32
NP_BF16 = ml_dtypes.bfloat16

B, N, M = 4, 8192, 8192
N_CORES = 8
NC_N = N // 2  # 4096 rows per core
K_AUG = 33
TILES = NC_N // 128  # 32

W = 512  # window columns per 128-row tile (multiple of 256)
WC = W // 128  # window chunks
LOC_M = (TILES - 1) * 128 + W  # local column space actually covered
COL_G = 512  # columns per gpsimd partition-reduce call
N_GROUPS = -(-LOC_M // COL_G)
LOC_PAD = N_GROUPS * COL_G  # padded local column space
PAD_X = 64.0  # x-coord of padding points (far away; d2 ~ 4e3)
INIT_F16 = -65504.0  # colacc init (negated-distance identity for max)
CHAIN_G = 8  # tiles per deferred row-min fold group


def build_program():
    nc = bacc.Bacc()

    lhs_d = nc.dram_tensor("lhs", [K_AUG, NC_N], BF16, kind="ExternalInput").ap()
    rhs_d = nc.dram_tensor("rhs", [K_AUG, LOC_PAD], BF16, kind="ExternalInput").ap()
    rowmin_d = nc.dram_tensor("rowmin", [128, TILES], FP32, kind="ExternalOutput").ap()
    colmin_d = nc.dram_tensor("colmin", [1, LOC_PAD], FP16, kind="ExternalOutput").ap()

    amax = mybir.AluOpType.max
    HW = W // 2

    with tile.TileContext(nc) as tc:
        with (
            tc.tile_pool(name="const", bufs=1) as const_pool,
            tc.tile_pool(name="acc", bufs=1) as acc_pool,
            tc.tile_pool(name="drain", bufs=3) as drain_pool,
            tc.tile_pool(name="out", bufs=1) as out_pool,
            tc.tile_pool(name="mm", bufs=3, space="PSUM") as mm_pool,
        ):
            lhs_sb = const_pool.tile([K_AUG, NC_N], BF16)
            rhs_sb = const_pool.tile([K_AUG, LOC_PAD], BF16)
            nc.sync.dma_start(out=lhs_sb, in_=lhs_d)
            nc.sync.dma_start(out=rhs_sb, in_=rhs_d)

            colacc = acc_pool.tile([128, LOC_PAD], FP16)
            rowhalf = acc_pool.tile([128, TILES * HW], FP16)
            colred = acc_pool.tile([128, LOC_PAD], FP16)
            rowmin_sb = out_pool.tile([128, TILES], FP32)

            # init col accumulator (GpSimd; first window's region first so
            # tile 0 can start folding early)
            nc.gpsimd.memset(colacc[:, :W], INIT_F16)
            nc.gpsimd.memset(colacc[:, W:], INIT_F16)

            # [128, n_chunk_pairs, 2, 128]: chunk parity as an explicit axis
            cv2 = colacc.rearrange("p (g2 pr c) -> p g2 pr c", pr=2, c=128)
            rhv = rowhalf.rearrange("p (t c) -> p t c", c=HW)
            rm = rowmin_sb.rearrange("p (t o) -> p t o", o=1)

            n_col_emitted = 0
            n_chain_emitted = 0

            for q in range(TILES // 2):  # tile pairs
                psum_t = mm_pool.tile([128, 2 * W], FP32, tag="mm")
                for u in range(2):
                    t = 2 * q + u
                    lhs_i = lhs_sb[:, t * 128 : (t + 1) * 128]
                    for v in range(W // 512):
                        c0 = t * 128 + v * 512
                        nc.tensor.matmul(
                            psum_t[:, u * W + v * 512 : u * W + (v + 1) * 512],
                            lhs_i,
                            rhs_sb[:, c0 : c0 + 512],
                        )
                drain = drain_pool.tile([128, 2 * W], FP16)
                nc.scalar.mul(drain, psum_t, -1.0)  # drain = -d2

                # first row-direction halving for both tiles in one op
                dvp = drain.rearrange("p (u c) -> p u c", u=2)
                nc.vector.tensor_tensor(
                    rhv[:, 2 * q : 2 * q + 2, :],
                    dvp[:, :, :HW],
                    dvp[:, :, HW:],
                    amax,
                )

                # col-direction folds (one strided op per tile over its
                # window chunks, split by absolute chunk parity)
                for u in range(2):
                    t = 2 * q + u
                    dwin2 = drain[:, u * W : (u + 1) * W].rearrange(
                        "p (g2 pr c) -> p g2 pr c", pr=2, c=128
                    )
                    for par in (0, 1):
                        off = (par - t) % 2  # first window chunk of this parity
                        g2a = (t + off) // 2  # absolute pair index of first
                        npair = WC // 2
                        sl = cv2[:, g2a : g2a + npair, par : par + 1, :]
                        nc.vector.tensor_tensor(
                            sl, sl, dwin2[:, :, off : off + 1, :], amax
                        )

                # partition-direction (col-min) reduce on GpSimd once a
                # COL_G region is final (last touch: tile cc); 4-tile margin
                while (
                    n_col_emitted < N_GROUPS
                    and min(
                        ((n_col_emitted + 1) * COL_G - 1) // 128, TILES - 1
                    )
                    <= 2 * q + 1 - 4
                ):
                    g = n_col_emitted
                    nc.gpsimd.partition_all_reduce(
                        colred[:, g * COL_G : (g + 1) * COL_G],
                        colacc[:, g * COL_G : (g + 1) * COL_G],
                        128,
                        bass_isa.ReduceOp.max,
                    )
                    n_col_emitted += 1

                # deferred row-direction fold chains per CHAIN_G tiles
                while (n_chain_emitted + 1) * CHAIN_G <= 2 * q + 2:
                    j = n_chain_emitted
                    seg = rhv[:, j * CHAIN_G : (j + 1) * CHAIN_G, :]
                    k = HW // 2
                    while k >= 2:
                        nc.vector.tensor_tensor(
                            seg[:, :, :k], seg[:, :, :k], seg[:, :, k : 2 * k], amax
                        )
                        k //= 2
                    nc.vector.tensor_tensor(
                        rm[:, j * CHAIN_G : (j + 1) * CHAIN_G, :],
                        seg[:, :, 0:1],
                        seg[:, :, 1:2],
                        amax,
                    )
                    n_chain_emitted += 1

            while n_col_emitted < N_GROUPS:
                g = n_col_emitted
                nc.gpsimd.partition_all_reduce(
                    colred[:, g * COL_G : (g + 1) * COL_G],
                    colacc[:, g * COL_G : (g + 1) * COL_G],
                    128,
                    bass_isa.ReduceOp.max,
                )
                n_col_emitted += 1

            nc.sync.dma_start(out=rowmin_d, in_=rowmin_sb)
            nc.sync.dma_start(out=colmin_d, in_=colred[0:1, :])

    nc.compile()
    return nc


def _split3(v):
    """v (f64 array) -> (hi, mid, lo) bf16 with hi+mid+lo ~= v (~26-bit)."""
    v = v.astype(np.float64)
    hi = v.astype(NP_BF16)
    r1 = v - hi.astype(np.float64)
    mid = r1.astype(NP_BF16)
    lo = (r1 - mid.astype(np.float64)).astype(NP_BF16)
    return hi, mid, lo


def _make_core_inputs(pts1, pts2):
    """pts1 (NC_N,3), pts2 (LOC_PAD,3) f64 -> lhs [33,NC_N], rhs [33,LOC_PAD] bf16.

    Row pairing (lhs_k paired with rhs_k), ordered so PE partial sums cancel
    early: d2 = sq1 + sq2 - 2*x1.x2 with 3-level splits.
    """
    a1 = _split3(pts1)
    a2 = _split3(pts2)
    n2 = [(-2.0 * p.astype(np.float64)).astype(NP_BF16) for p in a2]  # exact *-2
    sq1 = (pts1 * pts1).sum(-1)
    sq2 = (pts2 * pts2).sum(-1)
    s1 = _split3(sq1)
    s2 = _split3(sq2)

    ones_n = np.ones(pts1.shape[0], NP_BF16)
    ones_m = np.ones(pts2.shape[0], NP_BF16)

    lhs_rows = []
    rhs_rows = []

    def add(l, r):
        lhs_rows.append(l)
        rhs_rows.append(r)

    # big terms first, interleaved for cancellation
    add(s1[0], ones_m)
    for d in range(3):
        add(a1[0][:, d], n2[0][:, d])  # hi*hi
    add(ones_n, s2[0])
    # mid-level terms
    add(s1[1], ones_m)
    add(ones_n, s2[1])
    for d in range(3):
        add(a1[0][:, d], n2[1][:, d])  # hi*mid
    for d in range(3):
        add(a1[1][:, d], n2[0][:, d])  # mid*hi
    for d in range(3):
        add(a1[1][:, d], n2[1][:, d])  # mid*mid
    # low-level terms
    add(s1[2], ones_m)
    add(ones_n, s2[2])
    for d in range(3):
        add(a1[0][:, d], n2[2][:, d])  # hi*lo
    for d in range(3):
        add(a1[2][:, d], n2[0][:, d])  # lo*hi
    for d in range(3):
        add(a1[1][:, d], n2[2][:, d])  # mid*lo
    for d in range(3):
        add(a1[2][:, d], n2[1][:, d])  # lo*mid
    for d in range(3):
        add(a1[2][:, d], n2[2][:, d])  # lo*lo

    lhs = np.ascontiguousarray(np.stack(lhs_rows))
    rhs = np.ascontiguousarray(np.stack(rhs_rows))
    assert lhs.shape == (K_AUG, NC_N) and rhs.shape == (K_AUG, LOC_PAD)
    return lhs, rhs


_CACHED_NC = None


def _get_nc():
    global _CACHED_NC
    if _CACHED_NC is None:
        _CACHED_NC = build_program()
    return _CACHED_NC


def _coverage_rows_for_cols(h, j_global):
    """For sorted col ranks j (array), rows covered by core-half h's windows.

    Returns (r_lo, r_hi) global sorted row ranks [r_lo, r_hi) covered; empty
    coverage gives r_lo >= r_hi.
    """
    loc = j_global + W // 2 - NC_N * h  # local column index
    t_lo = np.maximum((loc - W) // 128 + 1, 0)
    t_hi = np.minimum(loc // 128, TILES - 1)
    valid = (t_lo <= t_hi) & (loc >= 0) & (loc < LOC_M)
    r_lo = np.where(valid, NC_N * h + 128 * t_lo, 0)
    r_hi = np.where(valid, NC_N * h + 128 * t_hi + 128, 0)
    return r_lo, r_hi


def kernel(xyz1, xyz2, _return_timing=False, _trace=False):
    xyz1 = np.asarray(xyz1, dtype=np.float32)
    xyz2 = np.asarray(xyz2, dtype=np.float32)
    assert xyz1.shape == (B, N, 3) and xyz2.shape == (B, M, 3)

    xs1 = []
    xs2 = []
    in_maps = []
    for b in range(B):
        p = xyz1[b].astype(np.float64)
        g = xyz2[b].astype(np.float64)
        o1 = np.argsort(p[:, 0], kind="stable")
        o2 = np.argsort(g[:, 0], kind="stable")
        ps, gs = p[o1], g[o2]
        xs1.append(ps)
        xs2.append(gs)
        for h in range(2):
            rows = ps[h * NC_N : (h + 1) * NC_N]
            # local col l -> global sorted col l - W/2 + NC_N*h; pad outside
            l0 = -(W // 2) + NC_N * h
            cols = np.full((LOC_PAD, 3), 0.0, dtype=np.float64)
            cols[:, 0] = PAD_X
            gidx = np.arange(l0, l0 + LOC_PAD)
            sel = (gidx >= 0) & (gidx < M)
            cols[sel] = gs[gidx[sel]]
            lhs, rhs = _make_core_inputs(rows, cols)
            in_maps.append({"lhs": lhs, "rhs": rhs})

    nc = _get_nc()
    res = run_bass_kernel_spmd(
        nc, in_maps, core_ids=list(range(N_CORES)), trace=_trace
    )

    total = 0.0
    for b in range(B):
        ps, gs = xs1[b], xs2[b]
        x1, x2 = ps[:, 0], gs[:, 0]

        # ---- row mins (sorted order; device stores -d2) ----
        row_parts = []
        for h in range(2):
            r = res.results[2 * b + h]
            row_parts.append(
                -np.asarray(r["rowmin"]).astype(np.float64).T.reshape(-1)
            )
        min1_d2 = np.concatenate(row_parts)  # (8192,) sorted rank order
        min1 = np.sqrt(np.maximum(min1_d2, 0.0))

        # ---- col mins ----
        col_d2 = np.full(M, np.inf)
        for h in range(2):
            r = res.results[2 * b + h]
            loc = -np.asarray(r["colmin"]).astype(np.float64).reshape(-1)
            l = np.arange(LOC_PAD)
            gidx = l - W // 2 + NC_N * h
            sel = (l < LOC_M) & (gidx >= 0) & (gidx < M)
            np.minimum.at(col_d2, gidx[sel], loc[sel])
        min2 = np.sqrt(np.maximum(col_d2, 0.0))

        # ---- flag + exact fix: rows ----
        r_rank = np.arange(N)
        t = (r_rank % NC_N) // 128
        h_arr = r_rank // NC_N
        glo = t * 128 + NC_N * h_arr - W // 2
        ghi = glo + W
        c_lo = np.maximum(glo, 0)
        c_hi = np.minimum(ghi, M)
        gapL = np.where(c_lo > 0, x1 - x2[np.maximum(c_lo - 1, 0)], np.inf)
        gapR = np.where(c_hi < M, x2[np.minimum(c_hi, M - 1)] - x1, np.inf)
        gap = np.maximum(np.minimum(gapL, gapR), 0.0)
        idx1 = np.where(min1 > gap * 0.999 - 1e-9)[0]
        if len(idx1):
            d2f = ((ps[idx1][:, None] - gs[None]) ** 2).sum(-1).min(1)
            min1[idx1] = np.sqrt(d2f)

        # ---- flag + exact fix: cols ----
        j = np.arange(M)
        r0_lo, r0_hi = _coverage_rows_for_cols(0, j)
        r1_lo, r1_hi = _coverage_rows_for_cols(1, j)
        # union of [r0_lo,r0_hi) and [r1_lo,r1_hi); empty segments excluded
        e0 = r0_hi > r0_lo
        e1 = r1_hi > r1_lo
        lo_all = np.where(e0, r0_lo, r1_lo)
        hi_all = np.where(e1, r1_hi, r0_hi)
        gapLc = np.where(lo_all > 0, x2 - x1[np.maximum(lo_all - 1, 0)], np.inf)
        gapRc = np.where(hi_all < N, x1[np.minimum(hi_all, N - 1)] - x2, np.inf)
        # middle gap when both segments exist and don't abut
        mid_gap = np.full(M, np.inf)
        mid = e0 & e1 & (r0_hi < r1_lo)
        if mid.any():
            a = np.abs(x1[np.minimum(r0_hi, N - 1)] - x2)
            bb = np.abs(x1[np.maximum(r1_lo - 1, 0)] - x2)
            mid_gap = np.where(mid, np.minimum(a, bb), np.inf)
        gapc = np.maximum(np.minimum(np.minimum(gapLc, gapRc), mid_gap), 0.0)
        idx2 = np.where(min2 > gapc * 0.999 - 1e-9)[0]
        if len(idx2):
            d2f = ((gs[idx2][:, None] - ps[None]) ** 2).sum(-1).min(1)
            min2[idx2] = np.sqrt(d2f)

        total += min1.mean() + min2.mean()

    out = np.asarray(total / B, dtype=np.float32)
    if _return_timing:
        return out, res
    return out


# revision 12
# speedup vs baseline: 6.8166x; 1.2100x over previous
"""Chamfer distance (L1) Trainium2 Bass kernel — sorted-window version.

Problem: xyz1 (4, 8192, 3) fp32, xyz2 (4, 8192, 3) fp32 ->
scalar = mean_b[ mean_n min_m ||x1-x2|| + mean_m min_n ||x1-x2|| ].

Strategy:
 - 8 cores: core c handles batch b=c//2, N-half h=c%2 (4096 rows).
 - Host sorts both clouds by x. A 128-row tile of sorted rows only needs
   distances against a W-column rank-window around its aligned position:
   the device computes a banded (windowed) distance matrix instead of the
   full 4096x8192 block — W/8192 of the dense work.
 - Rigorous exactness: a windowed row/col min is provably exact whenever
   it is <= the x-gap to the nearest excluded column/row (|dx| lower-bounds
   the distance). The host flags the few points (~0.5%) violating this and
   recomputes them exactly in numpy. Result: exact up to fp16 rounding.
 - d2[n,m] = ||x1n||^2 + ||x2m||^2 - 2 x1n.x2m as ONE matmul with K=33 rows
   of 3-level split-precision bf16 (~fp32 accuracy at bf16 PE speed).
 - All minimums run as MAX over NEGATED distances (ACT drains PSUM with a
   free *-1): max-folds on DVE for both directions.
 - Per 2-tile pair: PE matmuls -> PSUM [128,2W]; ACT drains -d2 to fp16;
   DVE: one [128,2,W/2] max-halving into a wide rowhalf buffer (log-folded
   in deferred groups) + one contiguous max-fold into colacc per tile.
 - Col-min finish: PE transposes final colacc chunks (interleaved into the
   main loop), DVE free-axis max-reduce; GpSimd only memsets colacc.
 - sqrt + means + flag-fix on host over ~12K values per core.
"""

import sys

sys.path.insert(0, "/opt/trn_rl_repo")

import numpy as np
import ml_dtypes

import concourse.bass as bass
import concourse.bass_isa as bass_isa
import concourse.bacc as bacc
import concourse.mybir as mybir
import concourse.tile as tile
from concourse.bass_utils import run_bass_kernel_spmd

BF16 = mybir.dt.bfloat16
FP16 = mybir.dt.float16
FP32 = mybir.dt.float32
NP_BF16 = ml_dtypes.bfloat16

B, N, M = 4, 8192, 8192
N_CORES = 8
NC_N = N // 2  # 4096 rows per core
K_AUG = 33
TILES = NC_N // 128  # 32

W = 512  # window columns per 128-row tile (multiple of 256)
WC = W // 128  # window chunks
LOC_M = (TILES - 1) * 128 + W  # local column space actually covered
TAIL_G = 4  # colacc chunks per tail transpose group
N_GROUPS = -(-(-(-LOC_M // 128)) // TAIL_G)
LOC_PAD = N_GROUPS * TAIL_G * 128  # padded local column space
PAD_X = 64.0  # x-coord of padding points (far away; d2 ~ 4e3)
INIT_F16 = -65504.0  # colacc init (negated-distance identity for max)
CHAIN_G = 8  # tiles per deferred row-min fold group


def build_program():
    nc = bacc.Bacc()

    lhs_d = nc.dram_tensor("lhs", [K_AUG, NC_N], BF16, kind="ExternalInput").ap()
    rhs_d = nc.dram_tensor("rhs", [K_AUG, LOC_PAD], BF16, kind="ExternalInput").ap()
    ident_d = nc.dram_tensor("ident", [128, 128], FP16, kind="ExternalInput").ap()
    rowmin_d = nc.dram_tensor("rowmin", [128, TILES], FP32, kind="ExternalOutput").ap()
    colmin_d = nc.dram_tensor(
        "colmin", [128, N_GROUPS * TAIL_G], FP32, kind="ExternalOutput"
    ).ap()

    amax = mybir.AluOpType.max
    ax_x = mybir.AxisListType.X
    HW = W // 2

    with tile.TileContext(nc) as tc:
        with (
            tc.tile_pool(name="const", bufs=1) as const_pool,
            tc.tile_pool(name="acc", bufs=1) as acc_pool,
            tc.tile_pool(name="drain", bufs=4) as drain_pool,
            tc.tile_pool(name="out", bufs=1) as out_pool,
            tc.tile_pool(name="mm", bufs=3, space="PSUM") as mm_pool,
            tc.tile_pool(name="tr", bufs=2, space="PSUM") as tr_pool,
        ):
            lhs_sb = const_pool.tile([K_AUG, NC_N], BF16)
            rhs_sb = const_pool.tile([K_AUG, LOC_PAD], BF16)
            ident_sb = const_pool.tile([128, 128], FP16)
            # split input DMAs so the first tiles' slices land first
            nc.sync.dma_start(out=ident_sb, in_=ident_d)
            nc.sync.dma_start(out=rhs_sb[:, :1024], in_=rhs_d[:, :1024])
            nc.sync.dma_start(out=lhs_sb[:, :512], in_=lhs_d[:, :512])
            nc.sync.dma_start(out=rhs_sb[:, 1024:2048], in_=rhs_d[:, 1024:2048])
            nc.sync.dma_start(out=lhs_sb[:, 512:], in_=lhs_d[:, 512:])
            nc.sync.dma_start(out=rhs_sb[:, 2048:], in_=rhs_d[:, 2048:])

            colacc = acc_pool.tile([128, LOC_PAD], FP16)
            rowhalf = acc_pool.tile([128, TILES * HW], FP16)
            rowmin_sb = out_pool.tile([128, TILES], FP32)
            colmin_sb = out_pool.tile([128, N_GROUPS * TAIL_G], FP32)

            # init col accumulator (GpSimd; first window's region first so
            # tile 0 can start folding early)
            nc.gpsimd.memset(colacc[:, :W], INIT_F16)
            nc.gpsimd.memset(colacc[:, W : 4 * W], INIT_F16)
            nc.gpsimd.memset(colacc[:, 4 * W :], INIT_F16)

            rhv = rowhalf.rearrange("p (t c) -> p t c", c=HW)
            rm = rowmin_sb.rearrange("p (t o) -> p t o", o=1)

            n_tail_emitted = 0
            n_chain_emitted = 0

            def emit_tail(g):
                tr_t = tr_pool.tile([128, TAIL_G * 128], FP16, tag="tr")
                for c4 in range(TAIL_G):
                    cc = g * TAIL_G + c4
                    nc.tensor.transpose(
                        tr_t[:, c4 * 128 : (c4 + 1) * 128],
                        colacc[:, cc * 128 : (cc + 1) * 128],
                        ident_sb,
                    )
                nc.vector.tensor_reduce(
                    colmin_sb[:, g * TAIL_G : (g + 1) * TAIL_G],
                    tr_t.rearrange("p (a b) -> p a b", b=128),
                    axis=ax_x,
                    op=amax,
                )

            for q in range(TILES // 2):  # tile pairs
                psum_t = mm_pool.tile([128, 2 * W], FP32, tag="mm")
                for u in range(2):
                    t = 2 * q + u
                    lhs_i = lhs_sb[:, t * 128 : (t + 1) * 128]
                    for v in range(W // 512):
                        c0 = t * 128 + v * 512
                        nc.tensor.matmul(
                            psum_t[:, u * W + v * 512 : u * W + (v + 1) * 512],
                            lhs_i,
                            rhs_sb[:, c0 : c0 + 512],
                        )
                drain = drain_pool.tile([128, 2 * W], FP16)
                nc.scalar.mul(drain, psum_t, -1.0)  # drain = -d2

                # first row-direction halving for both tiles in one op
                dvp = drain.rearrange("p (u c) -> p u c", u=2)
                nc.vector.tensor_tensor(
                    rhv[:, 2 * q : 2 * q + 2, :],
                    dvp[:, :, :HW],
                    dvp[:, :, HW:],
                    amax,
                )

                # col-direction folds: one contiguous op per tile
                for u in range(2):
                    t = 2 * q + u
                    sl = colacc[:, t * 128 : t * 128 + W]
                    nc.vector.tensor_tensor(
                        sl, sl, drain[:, u * W : (u + 1) * W], amax
                    )

                # transpose+reduce col groups once final (last touch: tile
                # of last chunk); 4-tile margin for cross-engine slack
                while (
                    n_tail_emitted < N_GROUPS
                    and min((n_tail_emitted + 1) * TAIL_G - 1, TILES - 1)
                    <= 2 * q + 1 - 4
                ):
                    emit_tail(n_tail_emitted)
                    n_tail_emitted += 1

                # deferred row-direction fold chains per CHAIN_G tiles
                while (n_chain_emitted + 1) * CHAIN_G <= 2 * q + 2:
                    j = n_chain_emitted
                    seg = rhv[:, j * CHAIN_G : (j + 1) * CHAIN_G, :]
                    k = HW // 2
                    while k >= 2:
                        nc.vector.tensor_tensor(
                            seg[:, :, :k], seg[:, :, :k], seg[:, :, k : 2 * k], amax
                        )
                        k //= 2
                    nc.vector.tensor_tensor(
                        rm[:, j * CHAIN_G : (j + 1) * CHAIN_G, :],
                        seg[:, :, 0:1],
                        seg[:, :, 1:2],
                        amax,
                    )
                    n_chain_emitted += 1

            while n_tail_emitted < N_GROUPS:
                emit_tail(n_tail_emitted)
                n_tail_emitted += 1

            nc.sync.dma_start(out=rowmin_d, in_=rowmin_sb)
            nc.sync.dma_start(out=colmin_d, in_=colmin_sb)

    nc.compile()
    return nc


def _split3(v):
    """v (f64 array) -> (hi, mid, lo) bf16 with hi+mid+lo ~= v (~26-bit)."""
    v = v.astype(np.float64)
    hi = v.astype(NP_BF16)
    r1 = v - hi.astype(np.float64)
    mid = r1.astype(NP_BF16)
    lo = (r1 - mid.astype(np.float64)).astype(NP_BF16)
    return hi, mid, lo


def _make_core_inputs(pts1, pts2):
    """pts1 (NC_N,3), pts2 (LOC_PAD,3) f64 -> lhs [33,NC_N], rhs [33,LOC_PAD] bf16.

    Row pairing (lhs_k paired with rhs_k), ordered so PE partial sums cancel
    early: d2 = sq1 + sq2 - 2*x1.x2 with 3-level splits.
    """
    a1 = _split3(pts1)
    a2 = _split3(pts2)
    n2 = [(-2.0 * p.astype(np.float64)).astype(NP_BF16) for p in a2]  # exact *-2
    sq1 = (pts1 * pts1).sum(-1)
    sq2 = (pts2 * pts2).sum(-1)
    s1 = _split3(sq1)
    s2 = _split3(sq2)

    ones_n = np.ones(pts1.shape[0], NP_BF16)
    ones_m = np.ones(pts2.shape[0], NP_BF16)

    lhs_rows = []
    rhs_rows = []

    def add(l, r):
        lhs_rows.append(l)
        rhs_rows.append(r)

    # big terms first, interleaved for cancellation
    add(s1[0], ones_m)
    for d in range(3):
        add(a1[0][:, d], n2[0][:, d])  # hi*hi
    add(ones_n, s2[0])
    # mid-level terms
    add(s1[1], ones_m)
    add(ones_n, s2[1])
    for d in range(3):
        add(a1[0][:, d], n2[1][:, d])  # hi*mid
    for d in range(3):
        add(a1[1][:, d], n2[0][:, d])  # mid*hi
    for d in range(3):
        add(a1[1][:, d], n2[1][:, d])  # mid*mid
    # low-level terms
    add(s1[2], ones_m)
    add(ones_n, s2[2])
    for d in range(3):
        add(a1[0][:, d], n2[2][:, d])  # hi*lo
    for d in range(3):
        add(a1[2][:, d], n2[0][:, d])  # lo*hi
    for d in range(3):
        add(a1[1][:, d], n2[2][:, d])  # mid*lo
    for d in range(3):
        add(a1[2][:, d], n2[1][:, d])  # lo*mid
    for d in range(3):
        add(a1[2][:, d], n2[2][:, d])  # lo*lo

    lhs = np.ascontiguousarray(np.stack(lhs_rows))
    rhs = np.ascontiguousarray(np.stack(rhs_rows))
    assert lhs.shape == (K_AUG, NC_N) and rhs.shape == (K_AUG, LOC_PAD)
    return lhs, rhs


_CACHED_NC = None


def _get_nc():
    global _CACHED_NC
    if _CACHED_NC is None:
        _CACHED_NC = build_program()
    return _CACHED_NC


def _coverage_rows_for_cols(h, j_global):
    """For sorted col ranks j (array), rows covered by core-half h's windows.

    Returns (r_lo, r_hi) global sorted row ranks [r_lo, r_hi) covered; empty
    coverage gives r_lo >= r_hi.
    """
    loc = j_global + W // 2 - NC_N * h  # local column index
    t_lo = np.maximum((loc - W) // 128 + 1, 0)
    t_hi = np.minimum(loc // 128, TILES - 1)
    valid = (t_lo <= t_hi) & (loc >= 0) & (loc < LOC_M)
    r_lo = np.where(valid, NC_N * h + 128 * t_lo, 0)
    r_hi = np.where(valid, NC_N * h + 128 * t_hi + 128, 0)
    return r_lo, r_hi


def kernel(xyz1, xyz2, _return_timing=False, _trace=False):
    xyz1 = np.asarray(xyz1, dtype=np.float32)
    xyz2 = np.asarray(xyz2, dtype=np.float32)
    assert xyz1.shape == (B, N, 3) and xyz2.shape == (B, M, 3)

    ident = np.eye(128, dtype=np.float16)
    xs1 = []
    xs2 = []
    in_maps = []
    for b in range(B):
        p = xyz1[b].astype(np.float64)
        g = xyz2[b].astype(np.float64)
        o1 = np.argsort(p[:, 0], kind="stable")
        o2 = np.argsort(g[:, 0], kind="stable")
        ps, gs = p[o1], g[o2]
        xs1.append(ps)
        xs2.append(gs)
        for h in range(2):
            rows = ps[h * NC_N : (h + 1) * NC_N]
            # local col l -> global sorted col l - W/2 + NC_N*h; pad outside
            l0 = -(W // 2) + NC_N * h
            cols = np.full((LOC_PAD, 3), 0.0, dtype=np.float64)
            cols[:, 0] = PAD_X
            gidx = np.arange(l0, l0 + LOC_PAD)
            sel = (gidx >= 0) & (gidx < M)
            cols[sel] = gs[gidx[sel]]
            lhs, rhs = _make_core_inputs(rows, cols)
            in_maps.append({"lhs": lhs, "rhs": rhs, "ident": ident})

    nc = _get_nc()
    res = run_bass_kernel_spmd(
        nc, in_maps, core_ids=list(range(N_CORES)), trace=_trace
    )

    total = 0.0
    for b in range(B):
        ps, gs = xs1[b], xs2[b]
        x1, x2 = ps[:, 0], gs[:, 0]

        # ---- row mins (sorted order; device stores -d2) ----
        row_parts = []
        for h in range(2):
            r = res.results[2 * b + h]
            row_parts.append(
                -np.asarray(r["rowmin"]).astype(np.float64).T.reshape(-1)
            )
        min1_d2 = np.concatenate(row_parts)  # (8192,) sorted rank order
        min1 = np.sqrt(np.maximum(min1_d2, 0.0))

        # ---- col mins ----
        col_d2 = np.full(M, np.inf)
        for h in range(2):
            r = res.results[2 * b + h]
            loc = -np.asarray(r["colmin"]).astype(np.float64).T.reshape(-1)
            l = np.arange(LOC_PAD)
            gidx = l - W // 2 + NC_N * h
            sel = (l < LOC_M) & (gidx >= 0) & (gidx < M)
            np.minimum.at(col_d2, gidx[sel], loc[sel])
        min2 = np.sqrt(np.maximum(col_d2, 0.0))

        # ---- flag + exact fix: rows ----
        r_rank = np.arange(N)
        t = (r_rank % NC_N) // 128
        h_arr = r_rank // NC_N
        glo = t * 128 + NC_N * h_arr - W // 2
        ghi = glo + W
        c_lo = np.maximum(glo, 0)
        c_hi = np.minimum(ghi, M)
        gapL = np.where(c_lo > 0, x1 - x2[np.maximum(c_lo - 1, 0)], np.inf)
        gapR = np.where(c_hi < M, x2[np.minimum(c_hi, M - 1)] - x1, np.inf)
        gap = np.maximum(np.minimum(gapL, gapR), 0.0)
        idx1 = np.where(min1 > gap * 0.999 - 1e-9)[0]
        if len(idx1):
            d2f = ((ps[idx1][:, None] - gs[None]) ** 2).sum(-1).min(1)
            min1[idx1] = np.sqrt(d2f)

        # ---- flag + exact fix: cols ----
        j = np.arange(M)
        r0_lo, r0_hi = _coverage_rows_for_cols(0, j)
        r1_lo, r1_hi = _coverage_rows_for_cols(1, j)
        # union of [r0_lo,r0_hi) and [r1_lo,r1_hi); empty segments excluded
        e0 = r0_hi > r0_lo
        e1 = r1_hi > r1_lo
        lo_all = np.where(e0, r0_lo, r1_lo)
        hi_all = np.where(e1, r1_hi, r0_hi)
        gapLc = np.where(lo_all > 0, x2 - x1[np.maximum(lo_all - 1, 0)], np.inf)
        gapRc = np.where(hi_all < N, x1[np.minimum(hi_all, N - 1)] - x2, np.inf)
        # middle gap when both segments exist and don't abut
        mid_gap = np.full(M, np.inf)
        mid = e0 & e1 & (r0_hi < r1_lo)
        if mid.any():
            a = np.abs(x1[np.minimum(r0_hi, N - 1)] - x2)
            bb = np.abs(x1[np.maximum(r1_lo - 1, 0)] - x2)
            mid_gap = np.where(mid, np.minimum(a, bb), np.inf)
        gapc = np.maximum(np.minimum(np.minimum(gapLc, gapRc), mid_gap), 0.0)
        idx2 = np.where(min2 > gapc * 0.999 - 1e-9)[0]
        if len(idx2):
            d2f = ((gs[idx2][:, None] - ps[None]) ** 2).sum(-1).min(1)
            min2[idx2] = np.sqrt(d2f)

        total += min1.mean() + min2.mean()

    out = np.asarray(total / B, dtype=np.float32)
    if _return_timing:
        return out, res
    return out


# revision 14
# speedup vs baseline: 7.4836x; 1.0979x over previous
"""Chamfer distance (L1) Trainium2 Bass kernel — sorted-window version.

Problem: xyz1 (4, 8192, 3) fp32, xyz2 (4, 8192, 3) fp32 ->
scalar = mean_b[ mean_n min_m ||x1-x2|| + mean_m min_n ||x1-x2|| ].

Strategy:
 - 8 cores: core c handles batch b=c//2, N-half h=c%2 (4096 rows).
 - Host sorts both clouds by x. A 128-row tile of sorted rows only needs
   distances against a W-column rank-window around its aligned position:
   the device computes a banded (windowed) distance matrix instead of the
   full 4096x8192 block — W/8192 of the dense work.
 - Rigorous exactness: a windowed row/col min is provably exact whenever
   it is <= the x-gap to the nearest excluded column/row (|dx| lower-bounds
   the distance). The host flags the few points (~0.5%) violating this and
   recomputes them exactly in numpy. Result: exact up to fp16 rounding.
 - d2[n,m] = ||x1n||^2 + ||x2m||^2 - 2 x1n.x2m as ONE matmul with K=33 rows
   of 3-level split-precision bf16 (~fp32 accuracy at bf16 PE speed).
 - All minimums run as MAX over NEGATED distances (ACT drains PSUM with a
   free *-1): max-folds on DVE for both directions.
 - Per 2-tile pair: PE matmuls -> PSUM [128,2W]; ACT drains -d2 to fp16;
   DVE: one [128,2,W/2] max-halving into a wide rowhalf buffer (log-folded
   in deferred groups) + one contiguous max-fold into colacc per tile.
 - Col-min finish: PE transposes final colacc chunks (interleaved into the
   main loop), DVE free-axis max-reduce; GpSimd only memsets colacc.
 - sqrt + means + flag-fix on host over ~12K values per core.
"""

import sys

sys.path.insert(0, "/opt/trn_rl_repo")

import numpy as np
import ml_dtypes

import concourse.bass as bass
import concourse.bass_isa as bass_isa
import concourse.bacc as bacc
import concourse.mybir as mybir
import concourse.tile as tile
from concourse.bass_utils import run_bass_kernel_spmd

BF16 = mybir.dt.bfloat16
FP16 = mybir.dt.float16
FP32 = mybir.dt.float32
NP_BF16 = ml_dtypes.bfloat16

B, N, M = 4, 8192, 8192
N_CORES = 8
NC_N = N // 2  # 4096 rows per core
K_AUG = 33
TILES = NC_N // 128  # 32

W = 384  # window columns per 128-row tile (multiple of 128)
WC = W // 128  # window chunks
LOC_M = (TILES - 1) * 128 + W  # local column space actually covered
TAIL_G = 4  # colacc chunks per tail transpose group
N_GROUPS = -(-(-(-LOC_M // 128)) // TAIL_G)
LOC_PAD = N_GROUPS * TAIL_G * 128  # padded local column space
PAD_X = 64.0  # x-coord of padding points (far away; d2 ~ 4e3)
INIT_F16 = -65504.0  # colacc init (negated-distance identity for max)
CHAIN_G = 8  # tiles per deferred row-min fold group
PW = -(-W // 512) * 512  # per-tile psum width, PSUM-bank (512 fp32) aligned


def build_program():
    nc = bacc.Bacc()

    lhs_d = nc.dram_tensor("lhs", [K_AUG, NC_N], BF16, kind="ExternalInput").ap()
    rhs_d = nc.dram_tensor("rhs", [K_AUG, LOC_PAD], BF16, kind="ExternalInput").ap()
    ident_d = nc.dram_tensor("ident", [128, 128], FP16, kind="ExternalInput").ap()
    outm_d = nc.dram_tensor(
        "outm", [128, TILES + N_GROUPS * TAIL_G], FP16, kind="ExternalOutput"
    ).ap()

    amax = mybir.AluOpType.max
    ax_x = mybir.AxisListType.X
    HW = W // 2

    with tile.TileContext(nc) as tc:
        with (
            tc.tile_pool(name="const", bufs=1) as const_pool,
            tc.tile_pool(name="acc", bufs=1) as acc_pool,
            tc.tile_pool(name="drain", bufs=4) as drain_pool,
            tc.tile_pool(name="out", bufs=1) as out_pool,
            tc.tile_pool(name="mm", bufs=3, space="PSUM") as mm_pool,
            tc.tile_pool(name="tr", bufs=2, space="PSUM") as tr_pool,
        ):
            lhs_sb = const_pool.tile([K_AUG, NC_N], BF16)
            rhs_sb = const_pool.tile([K_AUG, LOC_PAD], BF16)
            ident_sb = const_pool.tile([128, 128], FP16)
            # split input DMAs so the first tiles' slices land first
            nc.sync.dma_start(out=rhs_sb[:, :512], in_=rhs_d[:, :512])
            nc.sync.dma_start(out=lhs_sb[:, :256], in_=lhs_d[:, :256])
            nc.sync.dma_start(out=ident_sb, in_=ident_d)
            nc.sync.dma_start(out=rhs_sb[:, 512:1536], in_=rhs_d[:, 512:1536])
            nc.sync.dma_start(out=lhs_sb[:, 256:], in_=lhs_d[:, 256:])
            nc.sync.dma_start(out=rhs_sb[:, 1536:], in_=rhs_d[:, 1536:])

            colacc = acc_pool.tile([128, LOC_PAD], FP16)
            rowhalf = acc_pool.tile([128, TILES * HW], FP16)
            outm_sb = out_pool.tile([128, TILES + N_GROUPS * TAIL_G], FP16)
            rowmin_sb = outm_sb[:, :TILES]
            colmin_sb = outm_sb[:, TILES:]

            # init col accumulator (GpSimd; first window's region first so
            # tile 0 can start folding early)
            nc.gpsimd.memset(colacc[:, :W], INIT_F16)
            nc.gpsimd.memset(colacc[:, W : 4 * W], INIT_F16)
            nc.gpsimd.memset(colacc[:, 4 * W :], INIT_F16)

            rhv = rowhalf.rearrange("p (t c) -> p t c", c=HW)
            rm = rowmin_sb.rearrange("p (t o) -> p t o", o=1)

            n_tail_emitted = 0
            n_chain_emitted = 0

            def emit_tail(g):
                tr_t = tr_pool.tile([128, TAIL_G * 128], FP16, tag="tr")
                for c4 in range(TAIL_G):
                    cc = g * TAIL_G + c4
                    nc.tensor.transpose(
                        tr_t[:, c4 * 128 : (c4 + 1) * 128],
                        colacc[:, cc * 128 : (cc + 1) * 128],
                        ident_sb,
                    )
                nc.vector.tensor_reduce(
                    colmin_sb[:, g * TAIL_G : (g + 1) * TAIL_G],
                    tr_t.rearrange("p (a b) -> p a b", b=128),
                    axis=ax_x,
                    op=amax,
                )

            for q in range(TILES // 2):  # tile pairs
                psum_t = mm_pool.tile([128, 2 * PW], FP32, tag="mm")
                for u in range(2):
                    t = 2 * q + u
                    lhs_i = lhs_sb[:, t * 128 : (t + 1) * 128]
                    splits = [512] * (W // 512) + ([W % 512] if W % 512 else [])
                    c0 = t * 128
                    o0 = u * PW  # bank-aligned: matmul must not straddle banks
                    for sz in splits:
                        nc.tensor.matmul(
                            psum_t[:, o0 : o0 + sz],
                            lhs_i,
                            rhs_sb[:, c0 : c0 + sz],
                        )
                        c0 += sz
                        o0 += sz
                drain = drain_pool.tile([128, 2 * PW], FP16)
                nc.scalar.mul(drain, psum_t, -1.0)  # drain = -d2

                # first row-direction halving for both tiles in one op
                # (cols [W:PW) of each half are pad garbage, never read)
                dvp = drain.rearrange("p (u c) -> p u c", u=2)
                nc.vector.tensor_tensor(
                    rhv[:, 2 * q : 2 * q + 2, :],
                    dvp[:, :, :HW],
                    dvp[:, :, HW : 2 * HW],
                    amax,
                )

                # col-direction folds: one contiguous op per tile
                for u in range(2):
                    t = 2 * q + u
                    sl = colacc[:, t * 128 : t * 128 + W]
                    nc.vector.tensor_tensor(
                        sl, sl, drain[:, u * PW : u * PW + W], amax
                    )

                # transpose+reduce col groups once final (last touch: tile
                # of last chunk); 4-tile margin for cross-engine slack
                while (
                    n_tail_emitted < N_GROUPS
                    and min((n_tail_emitted + 1) * TAIL_G - 1, TILES - 1)
                    <= 2 * q + 1 - 4
                ):
                    emit_tail(n_tail_emitted)
                    n_tail_emitted += 1

                # deferred row-direction fold chains per CHAIN_G tiles
                while (n_chain_emitted + 1) * CHAIN_G <= 2 * q + 2:
                    j = n_chain_emitted
                    seg = rhv[:, j * CHAIN_G : (j + 1) * CHAIN_G, :]
                    k = HW // 2
                    while k % 2 == 0 and k > 48:
                        nc.vector.tensor_tensor(
                            seg[:, :, :k], seg[:, :, :k], seg[:, :, k : 2 * k], amax
                        )
                        k //= 2
                    nc.vector.tensor_reduce(
                        rm[:, j * CHAIN_G : (j + 1) * CHAIN_G, :],
                        seg[:, :, : 2 * k],
                        axis=ax_x,
                        op=amax,
                    )
                    n_chain_emitted += 1

            while n_tail_emitted < N_GROUPS:
                emit_tail(n_tail_emitted)
                n_tail_emitted += 1

            nc.sync.dma_start(out=outm_d, in_=outm_sb)

    nc.compile()
    return nc


def _split3(v):
    """v (f64 array) -> (hi, mid, lo) bf16 with hi+mid+lo ~= v (~26-bit)."""
    v = v.astype(np.float64)
    hi = v.astype(NP_BF16)
    r1 = v - hi.astype(np.float64)
    mid = r1.astype(NP_BF16)
    lo = (r1 - mid.astype(np.float64)).astype(NP_BF16)
    return hi, mid, lo


def _make_core_inputs(pts1, pts2):
    """pts1 (NC_N,3), pts2 (LOC_PAD,3) f64 -> lhs [33,NC_N], rhs [33,LOC_PAD] bf16.

    Row pairing (lhs_k paired with rhs_k), ordered so PE partial sums cancel
    early: d2 = sq1 + sq2 - 2*x1.x2 with 3-level splits.
    """
    a1 = _split3(pts1)
    a2 = _split3(pts2)
    n2 = [(-2.0 * p.astype(np.float64)).astype(NP_BF16) for p in a2]  # exact *-2
    sq1 = (pts1 * pts1).sum(-1)
    sq2 = (pts2 * pts2).sum(-1)
    s1 = _split3(sq1)
    s2 = _split3(sq2)

    ones_n = np.ones(pts1.shape[0], NP_BF16)
    ones_m = np.ones(pts2.shape[0], NP_BF16)

    lhs_rows = []
    rhs_rows = []

    def add(l, r):
        lhs_rows.append(l)
        rhs_rows.append(r)

    # big terms first, interleaved for cancellation
    add(s1[0], ones_m)
    for d in range(3):
        add(a1[0][:, d], n2[0][:, d])  # hi*hi
    add(ones_n, s2[0])
    # mid-level terms
    add(s1[1], ones_m)
    add(ones_n, s2[1])
    for d in range(3):
        add(a1[0][:, d], n2[1][:, d])  # hi*mid
    for d in range(3):
        add(a1[1][:, d], n2[0][:, d])  # mid*hi
    for d in range(3):
        add(a1[1][:, d], n2[1][:, d])  # mid*mid
    # low-level terms
    add(s1[2], ones_m)
    add(ones_n, s2[2])
    for d in range(3):
        add(a1[0][:, d], n2[2][:, d])  # hi*lo
    for d in range(3):
        add(a1[2][:, d], n2[0][:, d])  # lo*hi
    for d in range(3):
        add(a1[1][:, d], n2[2][:, d])  # mid*lo
    for d in range(3):
        add(a1[2][:, d], n2[1][:, d])  # lo*mid
    for d in range(3):
        add(a1[2][:, d], n2[2][:, d])  # lo*lo

    lhs = np.ascontiguousarray(np.stack(lhs_rows))
    rhs = np.ascontiguousarray(np.stack(rhs_rows))
    assert lhs.shape == (K_AUG, NC_N) and rhs.shape == (K_AUG, LOC_PAD)
    return lhs, rhs


def _exact_min_d2(a, b):
    """a (k,3), b (n,3) f64 -> (k,) min squared distance via gemm identity."""
    sa = (a * a).sum(-1)[:, None]
    sb = (b * b).sum(-1)[None, :]
    return (sa + sb - 2.0 * (a @ b.T)).min(1)


_CACHED_NC = None


def _get_nc():
    global _CACHED_NC
    if _CACHED_NC is None:
        _CACHED_NC = build_program()
    return _CACHED_NC


def _coverage_rows_for_cols(h, j_global):
    """For sorted col ranks j (array), rows covered by core-half h's windows.

    Returns (r_lo, r_hi) global sorted row ranks [r_lo, r_hi) covered; empty
    coverage gives r_lo >= r_hi.
    """
    loc = j_global + W // 2 - NC_N * h  # local column index
    t_lo = np.maximum((loc - W) // 128 + 1, 0)
    t_hi = np.minimum(loc // 128, TILES - 1)
    valid = (t_lo <= t_hi) & (loc >= 0) & (loc < LOC_M)
    r_lo = np.where(valid, NC_N * h + 128 * t_lo, 0)
    r_hi = np.where(valid, NC_N * h + 128 * t_hi + 128, 0)
    return r_lo, r_hi


def kernel(xyz1, xyz2, _return_timing=False, _trace=False):
    xyz1 = np.asarray(xyz1, dtype=np.float32)
    xyz2 = np.asarray(xyz2, dtype=np.float32)
    assert xyz1.shape == (B, N, 3) and xyz2.shape == (B, M, 3)

    ident = np.eye(128, dtype=np.float16)
    xs1 = []
    xs2 = []
    in_maps = []
    for b in range(B):
        p = xyz1[b].astype(np.float64)
        g = xyz2[b].astype(np.float64)
        o1 = np.argsort(p[:, 0], kind="stable")
        o2 = np.argsort(g[:, 0], kind="stable")
        ps, gs = p[o1], g[o2]
        xs1.append(ps)
        xs2.append(gs)
        for h in range(2):
            rows = ps[h * NC_N : (h + 1) * NC_N]
            # local col l -> global sorted col l - W/2 + NC_N*h; pad outside
            l0 = -(W // 2) + NC_N * h
            cols = np.full((LOC_PAD, 3), 0.0, dtype=np.float64)
            cols[:, 0] = PAD_X
            gidx = np.arange(l0, l0 + LOC_PAD)
            sel = (gidx >= 0) & (gidx < M)
            cols[sel] = gs[gidx[sel]]
            lhs, rhs = _make_core_inputs(rows, cols)
            in_maps.append({"lhs": lhs, "rhs": rhs, "ident": ident})

    nc = _get_nc()
    res = run_bass_kernel_spmd(
        nc, in_maps, core_ids=list(range(N_CORES)), trace=_trace
    )

    total = 0.0
    for b in range(B):
        ps, gs = xs1[b], xs2[b]
        x1, x2 = ps[:, 0], gs[:, 0]

        # ---- row mins (sorted order; device stores -d2) ----
        row_parts = []
        for h in range(2):
            r = res.results[2 * b + h]
            row_parts.append(
                -np.asarray(r["outm"])[:, :TILES].astype(np.float64).T.reshape(-1)
            )
        min1_d2 = np.concatenate(row_parts)  # (8192,) sorted rank order
        min1 = np.sqrt(np.maximum(min1_d2, 0.0))

        # ---- col mins ----
        col_d2 = np.full(M, np.inf)
        for h in range(2):
            r = res.results[2 * b + h]
            loc = (
                -np.asarray(r["outm"])[:, TILES:].astype(np.float64).T.reshape(-1)
            )
            l = np.arange(LOC_PAD)
            gidx = l - W // 2 + NC_N * h
            sel = (l < LOC_M) & (gidx >= 0) & (gidx < M)
            np.minimum.at(col_d2, gidx[sel], loc[sel])
        min2 = np.sqrt(np.maximum(col_d2, 0.0))

        # ---- flag + exact fix: rows ----
        r_rank = np.arange(N)
        t = (r_rank % NC_N) // 128
        h_arr = r_rank // NC_N
        glo = t * 128 + NC_N * h_arr - W // 2
        ghi = glo + W
        c_lo = np.maximum(glo, 0)
        c_hi = np.minimum(ghi, M)
        gapL = np.where(c_lo > 0, x1 - x2[np.maximum(c_lo - 1, 0)], np.inf)
        gapR = np.where(c_hi < M, x2[np.minimum(c_hi, M - 1)] - x1, np.inf)
        gap = np.maximum(np.minimum(gapL, gapR), 0.0)
        idx1 = np.where(min1 > gap * 0.999 - 1e-9)[0]
        if len(idx1):
            min1[idx1] = np.sqrt(np.maximum(_exact_min_d2(ps[idx1], gs), 0.0))

        # ---- flag + exact fix: cols ----
        j = np.arange(M)
        r0_lo, r0_hi = _coverage_rows_for_cols(0, j)
        r1_lo, r1_hi = _coverage_rows_for_cols(1, j)
        # union of [r0_lo,r0_hi) and [r1_lo,r1_hi); empty segments excluded
        e0 = r0_hi > r0_lo
        e1 = r1_hi > r1_lo
        lo_all = np.where(e0, r0_lo, r1_lo)
        hi_all = np.where(e1, r1_hi, r0_hi)
        gapLc = np.where(lo_all > 0, x2 - x1[np.maximum(lo_all - 1, 0)], np.inf)
        gapRc = np.where(hi_all < N, x1[np.minimum(hi_all, N - 1)] - x2, np.inf)
        # middle gap when both segments exist and don't abut
        mid_gap = np.full(M, np.inf)
        mid = e0 & e1 & (r0_hi < r1_lo)
        if mid.any():
            a = np.abs(x1[np.minimum(r0_hi, N - 1)] - x2)
            bb = np.abs(x1[np.maximum(r1_lo - 1, 0)] - x2)
            mid_gap = np.where(mid, np.minimum(a, bb), np.inf)
        gapc = np.maximum(np.minimum(np.minimum(gapLc, gapRc), mid_gap), 0.0)
        idx2 = np.where(min2 > gapc * 0.999 - 1e-9)[0]
        if len(idx2):
            min2[idx2] = np.sqrt(np.maximum(_exact_min_d2(gs[idx2], ps), 0.0))

        total += min1.mean() + min2.mean()

    out = np.asarray(total / B, dtype=np.float32)
    if _return_timing:
        return out, res
    return out


# revision 15
# speedup vs baseline: 7.5690x; 1.0114x over previous
"""Chamfer distance (L1) Trainium2 Bass kernel — sorted-window version.

Problem: xyz1 (4, 8192, 3) fp32, xyz2 (4, 8192, 3) fp32 ->
scalar = mean_b[ mean_n min_m ||x1-x2|| + mean_m min_n ||x1-x2|| ].

Strategy:
 - 8 cores: core c handles batch b=c//2, N-half h=c%2 (4096 rows).
 - Host sorts both clouds by x. A 128-row tile of sorted rows only needs
   distances against a W-column rank-window around its aligned position:
   the device computes a banded (windowed) distance matrix instead of the
   full 4096x8192 block — W/8192 of the dense work.
 - Rigorous exactness: a windowed row/col min is provably exact whenever
   it is <= the x-gap to the nearest excluded column/row (|dx| lower-bounds
   the distance). The host flags the few points (~0.5%) violating this and
   recomputes them exactly in numpy. Result: exact up to fp16 rounding.
 - d2[n,m] = ||x1n||^2 + ||x2m||^2 - 2 x1n.x2m as ONE matmul with K=33 rows
   of 3-level split-precision bf16 (~fp32 accuracy at bf16 PE speed).
 - All minimums run as MAX over NEGATED distances (ACT drains PSUM with a
   free *-1): max-folds on DVE for both directions.
 - Per 2-tile pair: PE matmuls -> PSUM [128,2W]; ACT drains -d2 to fp16;
   DVE: one [128,2,W/2] max-halving into a wide rowhalf buffer (log-folded
   in deferred groups) + one contiguous max-fold into colacc per tile.
 - Col-min finish: PE transposes final colacc chunks (interleaved into the
   main loop), DVE free-axis max-reduce; GpSimd only memsets colacc.
 - sqrt + means + flag-fix on host over ~12K values per core.
"""

import sys

sys.path.insert(0, "/opt/trn_rl_repo")

import numpy as np
import ml_dtypes

import concourse.bass as bass
import concourse.bass_isa as bass_isa
import concourse.bacc as bacc
import concourse.mybir as mybir
import concourse.tile as tile
from concourse.bass_utils import run_bass_kernel_spmd

BF16 = mybir.dt.bfloat16
FP16 = mybir.dt.float16
FP32 = mybir.dt.float32
NP_BF16 = ml_dtypes.bfloat16

B, N, M = 4, 8192, 8192
N_CORES = 8
NC_N = N // 2  # 4096 rows per core
K_AUG = 33
TILES = NC_N // 128  # 32

W = 384  # window columns per 128-row tile (multiple of 128)
WC = W // 128  # window chunks
LOC_M = (TILES - 1) * 128 + W  # local column space actually covered
TAIL_G = 4  # colacc chunks per tail transpose group
N_GROUPS = -(-(-(-LOC_M // 128)) // TAIL_G)
LOC_PAD = N_GROUPS * TAIL_G * 128  # padded local column space
PAD_X = 64.0  # x-coord of padding points (far away; d2 ~ 4e3)
INIT_F16 = -65504.0  # colacc init (negated-distance identity for max)
CHAIN_G = 8  # tiles per deferred row-min fold group
PW = -(-W // 512) * 512  # per-tile psum width, PSUM-bank (512 fp32) aligned


def build_program():
    nc = bacc.Bacc()

    lhs_d = nc.dram_tensor("lhs", [K_AUG, NC_N], BF16, kind="ExternalInput").ap()
    rhs_d = nc.dram_tensor("rhs", [K_AUG, LOC_PAD], BF16, kind="ExternalInput").ap()
    ident_d = nc.dram_tensor("ident", [128, 128], FP16, kind="ExternalInput").ap()
    outm_d = nc.dram_tensor(
        "outm", [128, TILES + N_GROUPS * TAIL_G], FP16, kind="ExternalOutput"
    ).ap()

    amax = mybir.AluOpType.max
    ax_x = mybir.AxisListType.X
    HW = W // 2

    with tile.TileContext(nc) as tc:
        with (
            tc.tile_pool(name="const", bufs=1) as const_pool,
            tc.tile_pool(name="acc", bufs=1) as acc_pool,
            tc.tile_pool(name="drain", bufs=4) as drain_pool,
            tc.tile_pool(name="out", bufs=1) as out_pool,
            tc.tile_pool(name="mm", bufs=3, space="PSUM") as mm_pool,
            tc.tile_pool(name="tr", bufs=2, space="PSUM") as tr_pool,
        ):
            lhs_sb = const_pool.tile([K_AUG, NC_N], BF16)
            rhs_sb = const_pool.tile([K_AUG, LOC_PAD], BF16)
            ident_sb = const_pool.tile([128, 128], FP16)
            # split input DMAs so the first tiles' slices land first; issue
            # the first two from the ACT queue (its sequencer frees earliest)
            nc.scalar.dma_start(out=rhs_sb[:, :512], in_=rhs_d[:, :512])
            nc.scalar.dma_start(out=lhs_sb[:, :256], in_=lhs_d[:, :256])
            nc.sync.dma_start(out=rhs_sb[:, 512:1536], in_=rhs_d[:, 512:1536])
            nc.sync.dma_start(out=lhs_sb[:, 256:], in_=lhs_d[:, 256:])
            nc.sync.dma_start(out=rhs_sb[:, 1536:], in_=rhs_d[:, 1536:])
            nc.sync.dma_start(out=ident_sb, in_=ident_d)

            colacc = acc_pool.tile([128, LOC_PAD], FP16)
            rowhalf = acc_pool.tile([128, TILES * HW], FP16)
            outm_sb = out_pool.tile([128, TILES + N_GROUPS * TAIL_G], FP16)
            rowmin_sb = outm_sb[:, :TILES]
            colmin_sb = outm_sb[:, TILES:]

            # init col accumulator (GpSimd; first window's region first so
            # tile 0 can start folding early)
            nc.gpsimd.memset(colacc[:, :W], INIT_F16)
            nc.gpsimd.memset(colacc[:, W : 4 * W], INIT_F16)
            nc.gpsimd.memset(colacc[:, 4 * W :], INIT_F16)

            rhv = rowhalf.rearrange("p (t c) -> p t c", c=HW)
            rm = rowmin_sb.rearrange("p (t o) -> p t o", o=1)

            n_tail_emitted = 0
            n_chain_emitted = 0

            def emit_tail(g):
                tr_t = tr_pool.tile([128, TAIL_G * 128], FP16, tag="tr")
                for c4 in range(TAIL_G):
                    cc = g * TAIL_G + c4
                    nc.tensor.transpose(
                        tr_t[:, c4 * 128 : (c4 + 1) * 128],
                        colacc[:, cc * 128 : (cc + 1) * 128],
                        ident_sb,
                    )
                nc.vector.tensor_reduce(
                    colmin_sb[:, g * TAIL_G : (g + 1) * TAIL_G],
                    tr_t.rearrange("p (a b) -> p a b", b=128),
                    axis=ax_x,
                    op=amax,
                )

            for q in range(TILES // 2):  # tile pairs
                psum_t = mm_pool.tile([128, 2 * PW], FP32, tag="mm")
                for u in range(2):
                    t = 2 * q + u
                    lhs_i = lhs_sb[:, t * 128 : (t + 1) * 128]
                    splits = [512] * (W // 512) + ([W % 512] if W % 512 else [])
                    c0 = t * 128
                    o0 = u * PW  # bank-aligned: matmul must not straddle banks
                    for sz in splits:
                        nc.tensor.matmul(
                            psum_t[:, o0 : o0 + sz],
                            lhs_i,
                            rhs_sb[:, c0 : c0 + sz],
                        )
                        c0 += sz
                        o0 += sz
                drain = drain_pool.tile([128, 2 * PW], FP16)
                nc.scalar.mul(drain, psum_t, -1.0)  # drain = -d2

                # first row-direction halving for both tiles in one op
                # (cols [W:PW) of each half are pad garbage, never read)
                dvp = drain.rearrange("p (u c) -> p u c", u=2)
                nc.vector.tensor_tensor(
                    rhv[:, 2 * q : 2 * q + 2, :],
                    dvp[:, :, :HW],
                    dvp[:, :, HW : 2 * HW],
                    amax,
                )

                # col-direction folds: one contiguous op per tile
                for u in range(2):
                    t = 2 * q + u
                    sl = colacc[:, t * 128 : t * 128 + W]
                    nc.vector.tensor_tensor(
                        sl, sl, drain[:, u * PW : u * PW + W], amax
                    )

                # transpose+reduce col groups once final (last touch: tile
                # of last chunk); 4-tile margin for cross-engine slack
                while (
                    n_tail_emitted < N_GROUPS
                    and min((n_tail_emitted + 1) * TAIL_G - 1, TILES - 1)
                    <= 2 * q + 1 - 4
                ):
                    emit_tail(n_tail_emitted)
                    n_tail_emitted += 1

                # deferred row-direction fold chains per CHAIN_G tiles
                while (n_chain_emitted + 1) * CHAIN_G <= 2 * q + 2:
                    j = n_chain_emitted
                    seg = rhv[:, j * CHAIN_G : (j + 1) * CHAIN_G, :]
                    k = HW // 2
                    while k % 2 == 0 and k > 48:
                        nc.vector.tensor_tensor(
                            seg[:, :, :k], seg[:, :, :k], seg[:, :, k : 2 * k], amax
                        )
                        k //= 2
                    nc.vector.tensor_reduce(
                        rm[:, j * CHAIN_G : (j + 1) * CHAIN_G, :],
                        seg[:, :, : 2 * k],
                        axis=ax_x,
                        op=amax,
                    )
                    n_chain_emitted += 1

            while n_tail_emitted < N_GROUPS:
                emit_tail(n_tail_emitted)
                n_tail_emitted += 1

            nc.sync.dma_start(out=outm_d, in_=outm_sb)

    nc.compile()
    return nc


def _split3(v):
    """v (f64 array) -> (hi, mid, lo) bf16 with hi+mid+lo ~= v (~26-bit)."""
    v = v.astype(np.float64)
    hi = v.astype(NP_BF16)
    r1 = v - hi.astype(np.float64)
    mid = r1.astype(NP_BF16)
    lo = (r1 - mid.astype(np.float64)).astype(NP_BF16)
    return hi, mid, lo


def _make_core_inputs(pts1, pts2):
    """pts1 (NC_N,3), pts2 (LOC_PAD,3) f64 -> lhs [33,NC_N], rhs [33,LOC_PAD] bf16.

    Row pairing (lhs_k paired with rhs_k), ordered so PE partial sums cancel
    early: d2 = sq1 + sq2 - 2*x1.x2 with 3-level splits.
    """
    a1 = _split3(pts1)
    a2 = _split3(pts2)
    n2 = [(-2.0 * p.astype(np.float64)).astype(NP_BF16) for p in a2]  # exact *-2
    sq1 = (pts1 * pts1).sum(-1)
    sq2 = (pts2 * pts2).sum(-1)
    s1 = _split3(sq1)
    s2 = _split3(sq2)

    ones_n = np.ones(pts1.shape[0], NP_BF16)
    ones_m = np.ones(pts2.shape[0], NP_BF16)

    lhs_rows = []
    rhs_rows = []

    def add(l, r):
        lhs_rows.append(l)
        rhs_rows.append(r)

    # big terms first, interleaved for cancellation
    add(s1[0], ones_m)
    for d in range(3):
        add(a1[0][:, d], n2[0][:, d])  # hi*hi
    add(ones_n, s2[0])
    # mid-level terms
    add(s1[1], ones_m)
    add(ones_n, s2[1])
    for d in range(3):
        add(a1[0][:, d], n2[1][:, d])  # hi*mid
    for d in range(3):
        add(a1[1][:, d], n2[0][:, d])  # mid*hi
    for d in range(3):
        add(a1[1][:, d], n2[1][:, d])  # mid*mid
    # low-level terms
    add(s1[2], ones_m)
    add(ones_n, s2[2])
    for d in range(3):
        add(a1[0][:, d], n2[2][:, d])  # hi*lo
    for d in range(3):
        add(a1[2][:, d], n2[0][:, d])  # lo*hi
    for d in range(3):
        add(a1[1][:, d], n2[2][:, d])  # mid*lo
    for d in range(3):
        add(a1[2][:, d], n2[1][:, d])  # lo*mid
    for d in range(3):
        add(a1[2][:, d], n2[2][:, d])  # lo*lo

    lhs = np.ascontiguousarray(np.stack(lhs_rows))
    rhs = np.ascontiguousarray(np.stack(rhs_rows))
    assert lhs.shape == (K_AUG, NC_N) and rhs.shape == (K_AUG, LOC_PAD)
    return lhs, rhs


def _exact_min_d2(a, b):
    """a (k,3), b (n,3) f64 -> (k,) min squared distance via gemm identity."""
    sa = (a * a).sum(-1)[:, None]
    sb = (b * b).sum(-1)[None, :]
    return (sa + sb - 2.0 * (a @ b.T)).min(1)


_CACHED_NC = None


def _get_nc():
    global _CACHED_NC
    if _CACHED_NC is None:
        _CACHED_NC = build_program()
    return _CACHED_NC


def _coverage_rows_for_cols(h, j_global):
    """For sorted col ranks j (array), rows covered by core-half h's windows.

    Returns (r_lo, r_hi) global sorted row ranks [r_lo, r_hi) covered; empty
    coverage gives r_lo >= r_hi.
    """
    loc = j_global + W // 2 - NC_N * h  # local column index
    t_lo = np.maximum((loc - W) // 128 + 1, 0)
    t_hi = np.minimum(loc // 128, TILES - 1)
    valid = (t_lo <= t_hi) & (loc >= 0) & (loc < LOC_M)
    r_lo = np.where(valid, NC_N * h + 128 * t_lo, 0)
    r_hi = np.where(valid, NC_N * h + 128 * t_hi + 128, 0)
    return r_lo, r_hi


def kernel(xyz1, xyz2, _return_timing=False, _trace=False):
    xyz1 = np.asarray(xyz1, dtype=np.float32)
    xyz2 = np.asarray(xyz2, dtype=np.float32)
    assert xyz1.shape == (B, N, 3) and xyz2.shape == (B, M, 3)

    ident = np.eye(128, dtype=np.float16)
    xs1 = []
    xs2 = []
    in_maps = []
    for b in range(B):
        p = xyz1[b].astype(np.float64)
        g = xyz2[b].astype(np.float64)
        o1 = np.argsort(p[:, 0], kind="stable")
        o2 = np.argsort(g[:, 0], kind="stable")
        ps, gs = p[o1], g[o2]
        xs1.append(ps)
        xs2.append(gs)
        for h in range(2):
            rows = ps[h * NC_N : (h + 1) * NC_N]
            # local col l -> global sorted col l - W/2 + NC_N*h; pad outside
            l0 = -(W // 2) + NC_N * h
            cols = np.full((LOC_PAD, 3), 0.0, dtype=np.float64)
            cols[:, 0] = PAD_X
            gidx = np.arange(l0, l0 + LOC_PAD)
            sel = (gidx >= 0) & (gidx < M)
            cols[sel] = gs[gidx[sel]]
            lhs, rhs = _make_core_inputs(rows, cols)
            in_maps.append({"lhs": lhs, "rhs": rhs, "ident": ident})

    nc = _get_nc()
    res = run_bass_kernel_spmd(
        nc, in_maps, core_ids=list(range(N_CORES)), trace=_trace
    )

    total = 0.0
    for b in range(B):
        ps, gs = xs1[b], xs2[b]
        x1, x2 = ps[:, 0], gs[:, 0]

        # ---- row mins (sorted order; device stores -d2) ----
        row_parts = []
        for h in range(2):
            r = res.results[2 * b + h]
            row_parts.append(
                -np.asarray(r["outm"])[:, :TILES].astype(np.float64).T.reshape(-1)
            )
        min1_d2 = np.concatenate(row_parts)  # (8192,) sorted rank order
        min1 = np.sqrt(np.maximum(min1_d2, 0.0))

        # ---- col mins ----
        col_d2 = np.full(M, np.inf)
        for h in range(2):
            r = res.results[2 * b + h]
            loc = (
                -np.asarray(r["outm"])[:, TILES:].astype(np.float64).T.reshape(-1)
            )
            l = np.arange(LOC_PAD)
            gidx = l - W // 2 + NC_N * h
            sel = (l < LOC_M) & (gidx >= 0) & (gidx < M)
            np.minimum.at(col_d2, gidx[sel], loc[sel])
        min2 = np.sqrt(np.maximum(col_d2, 0.0))

        # ---- flag + exact fix: rows ----
        r_rank = np.arange(N)
        t = (r_rank % NC_N) // 128
        h_arr = r_rank // NC_N
        glo = t * 128 + NC_N * h_arr - W // 2
        ghi = glo + W
        c_lo = np.maximum(glo, 0)
        c_hi = np.minimum(ghi, M)
        gapL = np.where(c_lo > 0, x1 - x2[np.maximum(c_lo - 1, 0)], np.inf)
        gapR = np.where(c_hi < M, x2[np.minimum(c_hi, M - 1)] - x1, np.inf)
        gap = np.maximum(np.minimum(gapL, gapR), 0.0)
        idx1 = np.where(min1 > gap * 0.999 - 1e-9)[0]
        if len(idx1):
            min1[idx1] = np.sqrt(np.maximum(_exact_min_d2(ps[idx1], gs), 0.0))

        # ---- flag + exact fix: cols ----
        j = np.arange(M)
        r0_lo, r0_hi = _coverage_rows_for_cols(0, j)
        r1_lo, r1_hi = _coverage_rows_for_cols(1, j)
        # union of [r0_lo,r0_hi) and [r1_lo,r1_hi); empty segments excluded
        e0 = r0_hi > r0_lo
        e1 = r1_hi > r1_lo
        lo_all = np.where(e0, r0_lo, r1_lo)
        hi_all = np.where(e1, r1_hi, r0_hi)
        gapLc = np.where(lo_all > 0, x2 - x1[np.maximum(lo_all - 1, 0)], np.inf)
        gapRc = np.where(hi_all < N, x1[np.minimum(hi_all, N - 1)] - x2, np.inf)
        # middle gap when both segments exist and don't abut
        mid_gap = np.full(M, np.inf)
        mid = e0 & e1 & (r0_hi < r1_lo)
        if mid.any():
            a = np.abs(x1[np.minimum(r0_hi, N - 1)] - x2)
            bb = np.abs(x1[np.maximum(r1_lo - 1, 0)] - x2)
            mid_gap = np.where(mid, np.minimum(a, bb), np.inf)
        gapc = np.maximum(np.minimum(np.minimum(gapLc, gapRc), mid_gap), 0.0)
        idx2 = np.where(min2 > gapc * 0.999 - 1e-9)[0]
        if len(idx2):
            min2[idx2] = np.sqrt(np.maximum(_exact_min_d2(gs[idx2], ps), 0.0))

        total += min1.mean() + min2.mean()

    out = np.asarray(total / B, dtype=np.float32)
    if _return_timing:
        return out, res
    return out


# revision 16
# speedup vs baseline: 8.2632x; 1.0917x over previous
"""Chamfer distance (L1) Trainium2 Bass kernel — sorted-window version.

Problem: xyz1 (4, 8192, 3) fp32, xyz2 (4, 8192, 3) fp32 ->
scalar = mean_b[ mean_n min_m ||x1-x2|| + mean_m min_n ||x1-x2|| ].

Strategy:
 - 8 cores: core c handles batch b=c//2, N-half h=c%2 (4096 rows).
 - Host sorts both clouds by x. A 128-row tile of sorted rows only needs
   distances against a W-column rank-window around its aligned position:
   the device computes a banded (windowed) distance matrix instead of the
   full 4096x8192 block — W/8192 of the dense work.
 - Rigorous exactness: a windowed row/col min is provably exact whenever
   it is <= the x-gap to the nearest excluded column/row (|dx| lower-bounds
   the distance). The host flags the few points (~0.5%) violating this and
   recomputes them exactly in numpy. Result: exact up to fp16 rounding.
 - d2[n,m] = ||x1n||^2 + ||x2m||^2 - 2 x1n.x2m as ONE matmul with K=33 rows
   of 3-level split-precision bf16 (~fp32 accuracy at bf16 PE speed).
 - All minimums run as MAX over NEGATED distances (ACT drains PSUM with a
   free *-1): max-folds on DVE for both directions.
 - Per 2-tile pair: PE matmuls -> PSUM [128,2W]; ACT drains -d2 to fp16;
   DVE: one [128,2,W/2] max-halving into a wide rowhalf buffer (log-folded
   in deferred groups) + one contiguous max-fold into colacc per tile.
 - Col-min finish: PE transposes final colacc chunks (interleaved into the
   main loop), DVE free-axis max-reduce; GpSimd only memsets colacc.
 - sqrt + means + flag-fix on host over ~12K values per core.
"""

import sys

sys.path.insert(0, "/opt/trn_rl_repo")

import numpy as np
import ml_dtypes

import concourse.bass as bass
import concourse.bass_isa as bass_isa
import concourse.bacc as bacc
import concourse.mybir as mybir
import concourse.tile as tile
from concourse.bass_utils import run_bass_kernel_spmd

BF16 = mybir.dt.bfloat16
FP16 = mybir.dt.float16
FP32 = mybir.dt.float32
NP_BF16 = ml_dtypes.bfloat16

B, N, M = 4, 8192, 8192
N_CORES = 8
NC_N = N // 2  # 4096 rows per core
K_AUG = 33
TILES = NC_N // 128  # 32

W = 256  # window columns per 128-row tile (multiple of 128)
WC = W // 128  # window chunks
TPG = 2 if W >= 384 else 4  # tiles per psum/drain group
PW_ = W if 512 % W == 0 else -(-W // 512) * 512  # per-tile psum stride
LOC_M = (TILES - 1) * 128 + W  # local column space actually covered
TAIL_G = 4  # colacc chunks per tail transpose group
N_GROUPS = -(-(-(-LOC_M // 128)) // TAIL_G)
LOC_PAD = N_GROUPS * TAIL_G * 128  # padded local column space
PAD_X = 64.0  # x-coord of padding points (far away; d2 ~ 4e3)
INIT_F16 = -65504.0  # colacc init (negated-distance identity for max)
CHAIN_G = 8  # tiles per deferred row-min fold group
PW = PW_  # per-tile psum stride, PSUM-bank (512 fp32) safe


def build_program():
    nc = bacc.Bacc()

    lhs_d = nc.dram_tensor("lhs", [K_AUG, NC_N], BF16, kind="ExternalInput").ap()
    rhs_d = nc.dram_tensor("rhs", [K_AUG, LOC_PAD], BF16, kind="ExternalInput").ap()
    ident_d = nc.dram_tensor("ident", [128, 128], FP16, kind="ExternalInput").ap()
    outm_d = nc.dram_tensor(
        "outm", [128, TILES + N_GROUPS * TAIL_G], FP16, kind="ExternalOutput"
    ).ap()

    amax = mybir.AluOpType.max
    ax_x = mybir.AxisListType.X
    HW = W // 2

    with tile.TileContext(nc) as tc:
        with (
            tc.tile_pool(name="const", bufs=1) as const_pool,
            tc.tile_pool(name="acc", bufs=1) as acc_pool,
            tc.tile_pool(name="drain", bufs=4) as drain_pool,
            tc.tile_pool(name="out", bufs=1) as out_pool,
            tc.tile_pool(name="mm", bufs=3, space="PSUM") as mm_pool,
            tc.tile_pool(name="tr", bufs=2, space="PSUM") as tr_pool,
        ):
            lhs_sb = const_pool.tile([K_AUG, NC_N], BF16)
            rhs_sb = const_pool.tile([K_AUG, LOC_PAD], BF16)
            ident_sb = const_pool.tile([128, 128], FP16)
            # split input DMAs so the first tiles' slices land first; issue
            # the first two from the ACT queue (its sequencer frees earliest)
            nc.scalar.dma_start(out=rhs_sb[:, :768], in_=rhs_d[:, :768])
            nc.scalar.dma_start(out=lhs_sb[:, :512], in_=lhs_d[:, :512])
            nc.sync.dma_start(out=rhs_sb[:, 768:1792], in_=rhs_d[:, 768:1792])
            nc.sync.dma_start(out=lhs_sb[:, 512:], in_=lhs_d[:, 512:])
            nc.sync.dma_start(out=rhs_sb[:, 1792:], in_=rhs_d[:, 1792:])
            nc.sync.dma_start(out=ident_sb, in_=ident_d)

            colacc = acc_pool.tile([128, LOC_PAD], FP16)
            rowhalf = acc_pool.tile([128, TILES * HW], FP16)
            outm_sb = out_pool.tile([128, TILES + N_GROUPS * TAIL_G], FP16)
            rowmin_sb = outm_sb[:, :TILES]
            colmin_sb = outm_sb[:, TILES:]

            # init col accumulator (GpSimd; first window's region first so
            # tile 0 can start folding early)
            nc.gpsimd.memset(colacc[:, :W], INIT_F16)
            nc.gpsimd.memset(colacc[:, W : 4 * W], INIT_F16)
            nc.gpsimd.memset(colacc[:, 4 * W :], INIT_F16)

            rhv = rowhalf.rearrange("p (t c) -> p t c", c=HW)
            rm = rowmin_sb.rearrange("p (t o) -> p t o", o=1)

            n_tail_emitted = 0
            n_chain_emitted = 0

            def emit_tail(g):
                tr_t = tr_pool.tile([128, TAIL_G * 128], FP16, tag="tr")
                for c4 in range(TAIL_G):
                    cc = g * TAIL_G + c4
                    nc.tensor.transpose(
                        tr_t[:, c4 * 128 : (c4 + 1) * 128],
                        colacc[:, cc * 128 : (cc + 1) * 128],
                        ident_sb,
                    )
                nc.vector.tensor_reduce(
                    colmin_sb[:, g * TAIL_G : (g + 1) * TAIL_G],
                    tr_t.rearrange("p (a b) -> p a b", b=128),
                    axis=ax_x,
                    op=amax,
                )

            for q in range(TILES // TPG):  # tile groups
                psum_t = mm_pool.tile([128, TPG * PW], FP32, tag="mm")
                for u in range(TPG):
                    t = TPG * q + u
                    lhs_i = lhs_sb[:, t * 128 : (t + 1) * 128]
                    splits = [512] * (W // 512) + ([W % 512] if W % 512 else [])
                    c0 = t * 128
                    o0 = u * PW  # bank-aligned: matmul must not straddle banks
                    for sz in splits:
                        nc.tensor.matmul(
                            psum_t[:, o0 : o0 + sz],
                            lhs_i,
                            rhs_sb[:, c0 : c0 + sz],
                        )
                        c0 += sz
                        o0 += sz
                drain = drain_pool.tile([128, TPG * PW], FP16)
                nc.scalar.mul(drain, psum_t, -1.0)  # drain = -d2

                # first row-direction halving for all group tiles in one op
                # (cols [W:PW) of each slot are pad garbage, never read)
                dvp = drain.rearrange("p (u c) -> p u c", u=TPG)
                nc.vector.tensor_tensor(
                    rhv[:, TPG * q : TPG * (q + 1), :],
                    dvp[:, :, :HW],
                    dvp[:, :, HW : 2 * HW],
                    amax,
                )

                # col-direction folds: one contiguous op per tile
                for u in range(TPG):
                    t = TPG * q + u
                    sl = colacc[:, t * 128 : t * 128 + W]
                    nc.vector.tensor_tensor(
                        sl, sl, drain[:, u * PW : u * PW + W], amax
                    )

                # transpose+reduce col groups once final (last touch: tile
                # of last chunk); 4-tile margin for cross-engine slack
                while (
                    n_tail_emitted < N_GROUPS
                    and min((n_tail_emitted + 1) * TAIL_G - 1, TILES - 1)
                    <= TPG * (q + 1) - 1 - 4
                ):
                    emit_tail(n_tail_emitted)
                    n_tail_emitted += 1

                # deferred row-direction fold chains per CHAIN_G tiles
                while (n_chain_emitted + 1) * CHAIN_G <= TPG * (q + 1):
                    j = n_chain_emitted
                    seg = rhv[:, j * CHAIN_G : (j + 1) * CHAIN_G, :]
                    k = HW // 2
                    while k % 2 == 0 and k > 48:
                        nc.vector.tensor_tensor(
                            seg[:, :, :k], seg[:, :, :k], seg[:, :, k : 2 * k], amax
                        )
                        k //= 2
                    nc.vector.tensor_reduce(
                        rm[:, j * CHAIN_G : (j + 1) * CHAIN_G, :],
                        seg[:, :, : 2 * k],
                        axis=ax_x,
                        op=amax,
                    )
                    n_chain_emitted += 1

            while n_tail_emitted < N_GROUPS:
                emit_tail(n_tail_emitted)
                n_tail_emitted += 1

            nc.sync.dma_start(out=outm_d, in_=outm_sb)

    nc.compile()
    return nc


def _split3(v):
    """v (f64 array) -> (hi, mid, lo) bf16 with hi+mid+lo ~= v (~26-bit)."""
    v = v.astype(np.float64)
    hi = v.astype(NP_BF16)
    r1 = v - hi.astype(np.float64)
    mid = r1.astype(NP_BF16)
    lo = (r1 - mid.astype(np.float64)).astype(NP_BF16)
    return hi, mid, lo


def _make_core_inputs(pts1, pts2):
    """pts1 (NC_N,3), pts2 (LOC_PAD,3) f64 -> lhs [33,NC_N], rhs [33,LOC_PAD] bf16.

    Row pairing (lhs_k paired with rhs_k), ordered so PE partial sums cancel
    early: d2 = sq1 + sq2 - 2*x1.x2 with 3-level splits.
    """
    a1 = _split3(pts1)
    a2 = _split3(pts2)
    n2 = [(-2.0 * p.astype(np.float64)).astype(NP_BF16) for p in a2]  # exact *-2
    sq1 = (pts1 * pts1).sum(-1)
    sq2 = (pts2 * pts2).sum(-1)
    s1 = _split3(sq1)
    s2 = _split3(sq2)

    ones_n = np.ones(pts1.shape[0], NP_BF16)
    ones_m = np.ones(pts2.shape[0], NP_BF16)

    lhs_rows = []
    rhs_rows = []

    def add(l, r):
        lhs_rows.append(l)
        rhs_rows.append(r)

    # big terms first, interleaved for cancellation
    add(s1[0], ones_m)
    for d in range(3):
        add(a1[0][:, d], n2[0][:, d])  # hi*hi
    add(ones_n, s2[0])
    # mid-level terms
    add(s1[1], ones_m)
    add(ones_n, s2[1])
    for d in range(3):
        add(a1[0][:, d], n2[1][:, d])  # hi*mid
    for d in range(3):
        add(a1[1][:, d], n2[0][:, d])  # mid*hi
    for d in range(3):
        add(a1[1][:, d], n2[1][:, d])  # mid*mid
    # low-level terms
    add(s1[2], ones_m)
    add(ones_n, s2[2])
    for d in range(3):
        add(a1[0][:, d], n2[2][:, d])  # hi*lo
    for d in range(3):
        add(a1[2][:, d], n2[0][:, d])  # lo*hi
    for d in range(3):
        add(a1[1][:, d], n2[2][:, d])  # mid*lo
    for d in range(3):
        add(a1[2][:, d], n2[1][:, d])  # lo*mid
    for d in range(3):
        add(a1[2][:, d], n2[2][:, d])  # lo*lo

    lhs = np.ascontiguousarray(np.stack(lhs_rows))
    rhs = np.ascontiguousarray(np.stack(rhs_rows))
    assert lhs.shape == (K_AUG, NC_N) and rhs.shape == (K_AUG, LOC_PAD)
    return lhs, rhs


def _exact_min_d2(a, b):
    """a (k,3), b (n,3) f64 -> (k,) min squared distance via gemm identity."""
    sa = (a * a).sum(-1)[:, None]
    sb = (b * b).sum(-1)[None, :]
    return (sa + sb - 2.0 * (a @ b.T)).min(1)


_CACHED_NC = None


def _get_nc():
    global _CACHED_NC
    if _CACHED_NC is None:
        _CACHED_NC = build_program()
    return _CACHED_NC


def _coverage_rows_for_cols(h, j_global):
    """For sorted col ranks j (array), rows covered by core-half h's windows.

    Returns (r_lo, r_hi) global sorted row ranks [r_lo, r_hi) covered; empty
    coverage gives r_lo >= r_hi.
    """
    loc = j_global + W // 2 - NC_N * h  # local column index
    t_lo = np.maximum((loc - W) // 128 + 1, 0)
    t_hi = np.minimum(loc // 128, TILES - 1)
    valid = (t_lo <= t_hi) & (loc >= 0) & (loc < LOC_M)
    r_lo = np.where(valid, NC_N * h + 128 * t_lo, 0)
    r_hi = np.where(valid, NC_N * h + 128 * t_hi + 128, 0)
    return r_lo, r_hi


def kernel(xyz1, xyz2, _return_timing=False, _trace=False):
    xyz1 = np.asarray(xyz1, dtype=np.float32)
    xyz2 = np.asarray(xyz2, dtype=np.float32)
    assert xyz1.shape == (B, N, 3) and xyz2.shape == (B, M, 3)

    ident = np.eye(128, dtype=np.float16)
    xs1 = []
    xs2 = []
    in_maps = []
    for b in range(B):
        p = xyz1[b].astype(np.float64)
        g = xyz2[b].astype(np.float64)
        o1 = np.argsort(p[:, 0], kind="stable")
        o2 = np.argsort(g[:, 0], kind="stable")
        ps, gs = p[o1], g[o2]
        xs1.append(ps)
        xs2.append(gs)
        for h in range(2):
            rows = ps[h * NC_N : (h + 1) * NC_N]
            # local col l -> global sorted col l - W/2 + NC_N*h; pad outside
            l0 = -(W // 2) + NC_N * h
            cols = np.full((LOC_PAD, 3), 0.0, dtype=np.float64)
            cols[:, 0] = PAD_X
            gidx = np.arange(l0, l0 + LOC_PAD)
            sel = (gidx >= 0) & (gidx < M)
            cols[sel] = gs[gidx[sel]]
            lhs, rhs = _make_core_inputs(rows, cols)
            in_maps.append({"lhs": lhs, "rhs": rhs, "ident": ident})

    nc = _get_nc()
    res = run_bass_kernel_spmd(
        nc, in_maps, core_ids=list(range(N_CORES)), trace=_trace
    )

    total = 0.0
    for b in range(B):
        ps, gs = xs1[b], xs2[b]
        x1, x2 = ps[:, 0], gs[:, 0]

        # ---- row mins (sorted order; device stores -d2) ----
        row_parts = []
        for h in range(2):
            r = res.results[2 * b + h]
            row_parts.append(
                -np.asarray(r["outm"])[:, :TILES].astype(np.float64).T.reshape(-1)
            )
        min1_d2 = np.concatenate(row_parts)  # (8192,) sorted rank order
        min1 = np.sqrt(np.maximum(min1_d2, 0.0))

        # ---- col mins ----
        col_d2 = np.full(M, np.inf)
        for h in range(2):
            r = res.results[2 * b + h]
            loc = (
                -np.asarray(r["outm"])[:, TILES:].astype(np.float64).T.reshape(-1)
            )
            l = np.arange(LOC_PAD)
            gidx = l - W // 2 + NC_N * h
            sel = (l < LOC_M) & (gidx >= 0) & (gidx < M)
            np.minimum.at(col_d2, gidx[sel], loc[sel])
        min2 = np.sqrt(np.maximum(col_d2, 0.0))

        # ---- flag + exact fix: rows ----
        r_rank = np.arange(N)
        t = (r_rank % NC_N) // 128
        h_arr = r_rank // NC_N
        glo = t * 128 + NC_N * h_arr - W // 2
        ghi = glo + W
        c_lo = np.maximum(glo, 0)
        c_hi = np.minimum(ghi, M)
        gapL = np.where(c_lo > 0, x1 - x2[np.maximum(c_lo - 1, 0)], np.inf)
        gapR = np.where(c_hi < M, x2[np.minimum(c_hi, M - 1)] - x1, np.inf)
        gap = np.maximum(np.minimum(gapL, gapR), 0.0)
        idx1 = np.where(min1 > gap * 0.999 - 1e-9)[0]
        if len(idx1):
            min1[idx1] = np.sqrt(np.maximum(_exact_min_d2(ps[idx1], gs), 0.0))

        # ---- flag + exact fix: cols ----
        j = np.arange(M)
        r0_lo, r0_hi = _coverage_rows_for_cols(0, j)
        r1_lo, r1_hi = _coverage_rows_for_cols(1, j)
        # union of [r0_lo,r0_hi) and [r1_lo,r1_hi); empty segments excluded
        e0 = r0_hi > r0_lo
        e1 = r1_hi > r1_lo
        lo_all = np.where(e0, r0_lo, r1_lo)
        hi_all = np.where(e1, r1_hi, r0_hi)
        gapLc = np.where(lo_all > 0, x2 - x1[np.maximum(lo_all - 1, 0)], np.inf)
        gapRc = np.where(hi_all < N, x1[np.minimum(hi_all, N - 1)] - x2, np.inf)
        # middle gap when both segments exist and don't abut
        mid_gap = np.full(M, np.inf)
        mid = e0 & e1 & (r0_hi < r1_lo)
        if mid.any():
            a = np.abs(x1[np.minimum(r0_hi, N - 1)] - x2)
            bb = np.abs(x1[np.maximum(r1_lo - 1, 0)] - x2)
            mid_gap = np.where(mid, np.minimum(a, bb), np.inf)
        gapc = np.maximum(np.minimum(np.minimum(gapLc, gapRc), mid_gap), 0.0)
        idx2 = np.where(min2 > gapc * 0.999 - 1e-9)[0]
        if len(idx2):
            min2[idx2] = np.sqrt(np.maximum(_exact_min_d2(gs[idx2], ps), 0.0))

        total += min1.mean() + min2.mean()

    out = np.asarray(total / B, dtype=np.float32)
    if _return_timing:
        return out, res
    return out
